# revision 28
# baseline (speedup 1.0000x reference)
"""Trainium2 Bass kernel for 16-head causal self-attention with RoPE.

Problem: x:[2,2048,2048] -> MHA(wq,wk,wv,wo, causal mask, RoPE) -> [2,2048,2048].

Sharding (8 NeuronCores): core = b*4 + g, where b in {0,1} is the batch
(data parallel) and g in {0..3} is a head group of 4 heads (tensor parallel
over the 16 heads / 2048 channels: group g owns channels [g*512, (g+1)*512)).

v3 design (pipelined attention, ACT-instruction minimization):
  - All intermediates SBUF-resident in bf16 as in v2 (q/k RoPE'd [dh,S] per
    head, v [S,cw] tiles, ao [dh, 4*S]); weights/x pre-laid-out on host.
  - Phase B (attention) is restructured around the measured HW costs:
    ACT costs (N+352)/1.2 ns per instruction, so exp is issued once per
    *pair* of score tiles over a 2-bank PSUM tile [128,1024] (80 instead of
    160 activations).  Wide masks [128,512] per diagonal-row-offset zero
    both the causal triangle and the never-written psum garbage, letting the
    PV matmuls run full width and the denominator accumulate with plain
    elementwise adds (alternating DVE/Pool), reduced by one ones-matmul per
    (head, chunk) block.
  - The PE queue is kept busy through exp latency by software pipelining:
    the PV matmuls of score-group g are emitted after the score matmuls +
    exp of group g+1, and independent GEMM work (projection pairs of chunk
    qi+1, out-projection pairs of chunk qi-1) is injected at paced filler
    points between groups.
  - PSUM budget: psA(3 banks: projections, RoPE rotate, out-proj, denom)
    + psS(2x2 banks: score groups) + psO(1 bank: PV accumulator) = 8.
Host: out[b] = sum of the 4 group partials + bo.
"""

import math
import sys

sys.path.insert(0, "/opt/trn_rl_repo")

import numpy as np

N_CORES = 8
B, S, D = 2, 2048, 2048
H, DH = 16, 128
G = 4                 # head groups (tensor-parallel factor per batch)
HPG = H // G          # heads per group = 4
CW = HPG * DH         # channels per group = 512
NT = S // 128         # 16 d-tiles of the contraction dim
SC = 512              # free-dim chunk (one PSUM bank of fp32)
NQ = S // SC          # 4 s-chunks

_NC_CACHE: dict = {}


def build_attn_nc(iters: int = 1, phases: int = 3):
    """Build + compile the Bass module (same program for all 8 cores)."""
    import concourse.tile as tile
    from concourse import bacc, mybir

    f32 = mybir.dt.float32
    bf16 = mybir.dt.bfloat16
    AF = mybir.ActivationFunctionType
    SCALE = 1.0 / math.sqrt(DH)

    nc = bacc.Bacc("TRN2", target_bir_lowering=False, debug=False,
                   num_devices=N_CORES)

    # host-pre-laid-out inputs (see host_prep)
    xTq = nc.dram_tensor("xTq", [NQ, 128, NT, SC], bf16,
                         kind="ExternalInput").ap()
    wqb = nc.dram_tensor("wqb", [128, NT, CW], bf16, kind="ExternalInput").ap()
    wkb = nc.dram_tensor("wkb", [128, NT, CW], bf16, kind="ExternalInput").ap()
    wvb = nc.dram_tensor("wvb", [128, NT, CW], bf16, kind="ExternalInput").ap()
    wob = nc.dram_tensor("wob", [128, HPG, D], bf16, kind="ExternalInput").ap()
    # packed constants: cf32 = bvb | bq | bk, cb16 = PT | ones | mask0..3
    cf32d = nc.dram_tensor("cf32d", [128, CW + 2 * HPG], f32,
                           kind="ExternalInput").ap()
    cb16d = nc.dram_tensor("cb16d", [128, 256 + 4 * SC], bf16,
                           kind="ExternalInput").ap()
    cosq = nc.dram_tensor("cosq", [NQ, DH, SC], bf16,
                          kind="ExternalInput").ap()
    sinq = nc.dram_tensor("sinq", [NQ, DH, SC], bf16,
                          kind="ExternalInput").ap()

    out = nc.dram_tensor("out", [S, D], bf16, kind="ExternalOutput").ap()

    with tile.TileContext(nc) as tc:
        for it in range(iters):
            with tc.tile_pool(name="const", bufs=1) as cpool, \
                 tc.tile_pool(name="wts", bufs=1) as wpool, \
                 tc.tile_pool(name="perst", bufs=1) as ppool:
                # ---- persistent SBUF tensors --------------------------
                qT = [ppool.tile([DH, S], bf16, name=f"qT{h}_{it}",
                                 tag=f"qT{h}") for h in range(HPG)]
                kT = [ppool.tile([DH, S], bf16, name=f"kT{h}_{it}",
                                 tag=f"kT{h}") for h in range(HPG)]
                v_t = [ppool.tile([128, CW], bf16, name=f"v{t}_{it}",
                                  tag=f"v{t}") for t in range(NT)]
                aoT = ppool.tile([128, HPG * S], bf16, name=f"aoT_{it}",
                                 tag="aoT")

                # ---- startup DMAs (order = queue service order) -------
                w_sb = {}
                for nm in ("q", "k"):
                    w_sb[nm] = wpool.tile([128, NT, CW], bf16,
                                          name=f"w{nm}_{it}", tag=f"w{nm}")
                # interleaved ramped parts: first q-pair chain can start
                # ~2us in; wk arrives before the first k-pair chain ends
                for j0, j1 in ((0, 1), (1, 2), (2, 4)):
                    nc.scalar.dma_start(w_sb["q"][:, j0:j1, :],
                                        wqb[:, j0:j1, :])
                nc.scalar.dma_start(w_sb["k"][:, 0:2, :], wkb[:, 0:2, :])
                cb16 = cpool.tile([128, 256 + 4 * SC], bf16,
                                  name=f"cb16{it}", tag="cb16")
                nc.scalar.dma_start(cb16[:], cb16d[:])
                cf32 = cpool.tile([128, CW + 2 * HPG], f32,
                                  name=f"cf32{it}", tag="cf32")
                nc.scalar.dma_start(cf32[:], cf32d[:])
                pt_sb = cb16[:, 0:DH]
                ones_sb = cb16[:, 128:256]
                msk = [cb16[:, 256 + r * SC:256 + (r + 1) * SC]
                       for r in range(4)]
                bvb_sb = cf32[:, 0:CW]
                bq_sb = [cf32[:, CW + ct:CW + ct + 1] for ct in range(HPG)]
                bk_sb = [cf32[:, CW + HPG + ct:CW + HPG + ct + 1]
                         for ct in range(HPG)]
                for j0, j1 in ((4, 8), (8, 16)):
                    nc.scalar.dma_start(w_sb["q"][:, j0:j1, :],
                                        wqb[:, j0:j1, :])
                for j0, j1 in ((2, 4), (4, 8), (8, 16)):
                    nc.scalar.dma_start(w_sb["k"][:, j0:j1, :],
                                        wkb[:, j0:j1, :])
                w_sb["v"] = wpool.tile([128, NT, CW], bf16,
                                       name=f"wv_{it}", tag="wv")
                nc.scalar.dma_start(w_sb["v"][:], wvb[:])
                wo_sb = wpool.tile([128, HPG, D], bf16, name=f"wo{it}",
                                   tag="wo")
                nc.scalar.dma_start(wo_sb[:], wob[:])

                import contextlib
                with contextlib.ExitStack() as est:
                    xqpool = est.enter_context(
                        tc.tile_pool(name="xqp", bufs=2))
                    cspool = est.enter_context(
                        tc.tile_pool(name="csp", bufs=2))
                    prawp = est.enter_context(
                        tc.tile_pool(name="prawp", bufs=4))
                    wkp = est.enter_context(
                        tc.tile_pool(name="workA", bufs=2))
                    atpool = est.enter_context(
                        tc.tile_pool(name="atp", bufs=4))
                    accpool = est.enter_context(
                        tc.tile_pool(name="accp", bufs=2))
                    recpool = est.enter_context(
                        tc.tile_pool(name="recp", bufs=2))
                    outpool = est.enter_context(
                        tc.tile_pool(name="outp", bufs=4))
                    psA = est.enter_context(
                        tc.tile_pool(name="psA", bufs=2, space="PSUM"))
                    psX = est.enter_context(
                        tc.tile_pool(name="psX", bufs=2, space="PSUM"))
                    psS2 = est.enter_context(
                        tc.tile_pool(name="psS2", bufs=1, space="PSUM"))
                    psS1 = est.enter_context(
                        tc.tile_pool(name="psS1", bufs=1, space="PSUM"))
                    psO = est.enter_context(
                        tc.tile_pool(name="psO", bufs=1, space="PSUM"))

                    def load_chunk(qi):
                        xq = xqpool.tile([128, NT, SC], bf16,
                                         name=f"xq{qi}_{it}", tag="xq")
                        cos_c = cspool.tile([DH, SC], bf16,
                                            name=f"cos{qi}_{it}", tag="cos")
                        sin_c = cspool.tile([DH, SC], bf16,
                                            name=f"sin{qi}_{it}", tag="sin")
                        if qi == 0:
                            # fine ramped parts: first matmul starts early
                            for j0, j1 in ((0, 1), (1, 2), (2, 4), (4, 8),
                                           (8, 16)):
                                nc.sync.dma_start(xq[:, j0:j1, :],
                                                  xTq[qi][:, j0:j1, :])
                        else:
                            nc.sync.dma_start(xq[:], xTq[qi])
                        nc.sync.dma_start(cos_c[:], cosq[qi])
                        nc.sync.dma_start(sin_c[:], sinq[qi])
                        return xq, cos_c, sin_c

                    def a_qk_stream(nm, cp, qi, xq):
                        """Projection pair chain; returns praw tiles for the
                        rope tail (emitted at a later filler point so the PE
                        never waits on the DVE drain)."""
                        bias = bq_sb if nm == "q" else bk_sb
                        psa = psA.tile([128, SC], f32,
                                       name=f"ps{nm}{cp}_{qi}_{it}",
                                       tag="psA")
                        psb = psA.tile([128, SC], f32,
                                       name=f"ps{nm}{cp+1}_{qi}_{it}",
                                       tag="psA")
                        for d in range(NT):
                            nc.tensor.matmul(
                                psa[:],
                                w_sb[nm][:, d, cp * DH:(cp + 1) * DH],
                                xq[:, d, :],
                                start=(d == 0), stop=(d == NT - 1))
                            nc.tensor.matmul(
                                psb[:],
                                w_sb[nm][:, d, (cp + 1) * DH:(cp + 2) * DH],
                                xq[:, d, :],
                                start=(d == 0), stop=(d == NT - 1))
                        praws = []
                        for ct, ps in ((cp, psa), (cp + 1, psb)):
                            praw = prawp.tile([128, SC], bf16,
                                              name=f"praw{nm}{ct}_{qi}_{it}",
                                              tag="praw")
                            nc.vector.tensor_scalar_add(praw[:], ps[:],
                                                        bias[ct])
                            praws.append(praw)
                        return praws

                    def a_qk_rope(nm, cp, qi, praws, cos_c, sin_c):
                        qkT = qT if nm == "q" else kT
                        for i, ct in enumerate((cp, cp + 1)):
                            praw = praws[i]
                            psr = psX.tile([128, SC], f32,
                                           name=f"psr{nm}{ct}_{qi}_{it}",
                                           tag="psX")
                            nc.tensor.matmul(psr[:], pt_sb, praw[:],
                                             start=True, stop=True)
                            m1 = wkp.tile([128, SC], bf16,
                                          name=f"m1{nm}{ct}_{qi}_{it}",
                                          tag="m1")
                            nc.vector.tensor_mul(m1[:], praw[:], cos_c[:])
                            m2 = wkp.tile([128, SC], bf16,
                                          name=f"m2{nm}{ct}_{qi}_{it}",
                                          tag="m2")
                            nc.vector.tensor_mul(m2[:], psr[:], sin_c[:])
                            nc.gpsimd.tensor_add(
                                qkT[ct][:, qi * SC:(qi + 1) * SC],
                                m1[:], m2[:])

                    def a_v_pair(sp, qi, xq):
                        psa = psA.tile([128, SC], f32,
                                       name=f"psv{sp}_{qi}_{it}", tag="psA")
                        psb = psA.tile([128, SC], f32,
                                       name=f"psv{sp+1}_{qi}_{it}",
                                       tag="psA")
                        for d in range(NT):
                            nc.tensor.matmul(
                                psa[:],
                                xq[:, d, sp * 128:(sp + 1) * 128],
                                w_sb["v"][:, d, :],
                                start=(d == 0), stop=(d == NT - 1))
                            nc.tensor.matmul(
                                psb[:],
                                xq[:, d, (sp + 1) * 128:(sp + 2) * 128],
                                w_sb["v"][:, d, :],
                                start=(d == 0), stop=(d == NT - 1))
                        nc.vector.tensor_add(v_t[qi * 4 + sp][:], psa[:],
                                             bvb_sb)
                        nc.vector.tensor_add(v_t[qi * 4 + sp + 1][:],
                                             psb[:], bvb_sb)

                    def c_pair(st, dcp):
                        psa = psA.tile([128, SC], f32,
                                       name=f"op{st}{dcp}_{it}", tag="psA")
                        psb = psA.tile([128, SC], f32,
                                       name=f"op{st}{dcp+1}_{it}", tag="psA")
                        for h in range(HPG):
                            lhs = aoT[:, h * S + st * 128:
                                      h * S + (st + 1) * 128]
                            nc.tensor.matmul(
                                psa[:], lhs,
                                wo_sb[:, h, dcp * SC:(dcp + 1) * SC],
                                start=(h == 0), stop=(h == HPG - 1))
                            nc.tensor.matmul(
                                psb[:], lhs,
                                wo_sb[:, h, (dcp + 1) * SC:(dcp + 2) * SC],
                                start=(h == 0), stop=(h == HPG - 1))
                        # copies stay off the ACT queue (strict FIFO, jammed
                        # behind pending exps during phase B)
                        for dc, op in ((dcp, psa), (dcp + 1, psb)):
                            ot = outpool.tile([128, SC], bf16,
                                              name=f"ot{st}{dc}_{it}",
                                              tag="ot")
                            nc.vector.tensor_copy(ot[:], op[:])
                            nc.sync.dma_start(
                                out[st * 128:(st + 1) * 128,
                                    dc * SC:(dc + 1) * SC], ot[:])

                    def b_block(h, c):
                        """Attention for (head h, query chunk c); yields at
                        filler points (once per score group)."""
                        ntile = 4 * c + 4
                        ngrp = ntile // 2
                        oT = psO.tile([DH, SC], f32, name=f"oT{h}{c}_{it}",
                                      tag="oT")
                        acc = accpool.tile([128, SC], bf16,
                                           name=f"acc{h}{c}_{it}", tag="acc")

                        def emit_pv(at, ts):
                            for j, t_ in enumerate(ts):
                                n0 = max(t_ - 4 * c, 0) * 128
                                nc.tensor.matmul(
                                    oT[:, n0:],
                                    v_t[t_][:, h * DH:(h + 1) * DH],
                                    at[:, j * SC + n0:(j + 1) * SC],
                                    start=(t_ == 0), stop=(t_ == ntile - 1),
                                    skip_group_check=True)

                        # groups alternate (pair, single) so the two score
                        # pools (2-bank / 1-bank) strictly interleave
                        groups = []
                        t = 0
                        while t < ntile:
                            if len(groups) % 2 == 0 and t + 1 < ntile:
                                groups.append((t, t + 1))
                                t += 2
                            else:
                                groups.append((t,))
                                t += 1
                        pend = []
                        for g, ts in enumerate(groups):
                            w = len(ts)
                            if w == 2:
                                ss = psS2.tile([128, 2 * SC], f32,
                                               name=f"ss{h}{c}{g}_{it}",
                                               tag="ss2")
                            else:
                                ss = psS1.tile([128, SC], f32,
                                               name=f"ss{h}{c}{g}_{it}",
                                               tag="ss1")
                            at = atpool.tile([128, w * SC], bf16,
                                             name=f"at{h}{c}{g}_{it}",
                                             tag="at2" if w == 2 else "at1")
                            for j, t_ in enumerate(ts):
                                # full width even for diagonal tiles: keeps
                                # every at element a defined finite value
                                # (masked-out cols are zeroed after exp)
                                nc.tensor.matmul(
                                    ss[:, j * SC:(j + 1) * SC],
                                    kT[h][:, t_ * 128:(t_ + 1) * 128],
                                    qT[h][:, c * SC:(c + 1) * SC],
                                    start=True, stop=True)
                            nc.scalar.activation(at[:], ss[:], AF.Exp,
                                                 bias=0.0, scale=SCALE)
                            for j, t_ in enumerate(ts):
                                rr = t_ - 4 * c
                                if rr >= 0:
                                    nc.vector.tensor_mul(
                                        at[:, j * SC:(j + 1) * SC],
                                        at[:, j * SC:(j + 1) * SC],
                                        msk[rr])
                            if g == 0:
                                nc.vector.tensor_add(acc[:], at[:, 0:SC],
                                                     at[:, SC:2 * SC])
                            else:
                                for j in range(w):
                                    nc.vector.tensor_add(
                                        acc[:], acc[:],
                                        at[:, j * SC:(j + 1) * SC])
                            pend.append((at, ts))
                            if len(pend) > 2:
                                yield
                                emit_pv(*pend.pop(0))
                            elif g >= 1:
                                yield
                        while pend:
                            yield
                            emit_pv(*pend.pop(0))
                        dnp = psX.tile([128, SC], f32,
                                       name=f"dn{h}{c}_{it}", tag="psX")
                        nc.tensor.matmul(dnp[:], ones_sb, acc[:],
                                         start=True, stop=True)
                        rec = recpool.tile([128, SC], f32,
                                           name=f"rec{h}{c}_{it}", tag="rec")
                        nc.vector.reciprocal(rec[:], dnp[:])
                        nc.vector.tensor_mul(
                            aoT[:, h * S + c * SC:h * S + (c + 1) * SC],
                            oT[:], rec[:])

                    # ---- prologue: A(0), rope tails one stream behind --
                    xq, cos_c, sin_c = load_chunk(0)
                    pr_q0 = a_qk_stream("q", 0, 0, xq)
                    pr_k0 = a_qk_stream("k", 0, 0, xq)
                    a_qk_rope("q", 0, 0, pr_q0, cos_c, sin_c)
                    pr_q2 = a_qk_stream("q", 2, 0, xq)
                    a_qk_rope("k", 0, 0, pr_k0, cos_c, sin_c)
                    pr_k2 = a_qk_stream("k", 2, 0, xq)
                    a_qk_rope("q", 2, 0, pr_q2, cos_c, sin_c)
                    a_v_pair(0, 0, xq)
                    a_qk_rope("k", 2, 0, pr_k2, cos_c, sin_c)
                    a_v_pair(2, 0, xq)

                    # ---- main loop: B(qi) + fillers A(qi+1), C(qi-1) --
                    for qi in range(NQ):
                        a_fill, c_fill = [], []
                        if qi + 1 < NQ:
                            xq2, cos2, sin2 = load_chunk(qi + 1)
                            qi1 = qi + 1
                            for nm in ("q", "k"):
                                for cp in (0, 2):
                                    hold = {}

                                    def fs(nm=nm, cp=cp, hold=hold,
                                           qi1=qi1, xq2=xq2):
                                        hold["p"] = a_qk_stream(nm, cp,
                                                                qi1, xq2)

                                    def fr(nm=nm, cp=cp, hold=hold,
                                           qi1=qi1, cos2=cos2, sin2=sin2):
                                        a_qk_rope(nm, cp, qi1, hold["p"],
                                                  cos2, sin2)

                                    a_fill.append(fs)
                                    a_fill.append(fr)
                            for sp in (0, 2):
                                a_fill.append(
                                    lambda sp=sp, qi1=qi1, xq2=xq2:
                                    a_v_pair(sp, qi1, xq2))
                        # out-projection fillers: B(1)<-C(0), B(3)<-C(1,2)
                        # (B(2) has plenty of A(3) filler; B(3) has no A)
                        C_SRC = {1: (0,), 3: (1, 2)}
                        if phases >= 3:
                            for cc in C_SRC.get(qi, ()):
                                for st in range(4 * cc, 4 * cc + 4):
                                    for dcp in (0, 2):
                                        c_fill.append(
                                            lambda st=st, dcp=dcp: c_pair(
                                                st, dcp))
                        # interleave A and C fillers
                        fillers = []
                        na, ncf = len(a_fill), len(c_fill)
                        ia = ic = 0
                        for k_ in range(na + ncf):
                            if ia * max(ncf, 1) <= ic * max(na, 1) and \
                                    ia < na:
                                fillers.append(a_fill[ia]); ia += 1
                            elif ic < ncf:
                                fillers.append(c_fill[ic]); ic += 1
                            else:
                                fillers.append(a_fill[ia]); ia += 1

                        if phases >= 2:
                            total_pts = HPG * ((3, 5, 8, 11)[qi] + 1)
                            done = 0
                            pt = 0
                            for h in range(HPG):
                                for _ in b_block(h, qi):
                                    pt += 1
                                    want = len(fillers) * pt // total_pts
                                    while done < want:
                                        fillers[done]()
                                        done += 1
                            while done < len(fillers):
                                fillers[done]()
                                done += 1
                        else:
                            for fl in fillers:
                                fl()

                    # ---- tail: out-projection for the last chunk ------
                    if phases >= 3:
                        for st in range(4 * (NQ - 1), 4 * NQ):
                            for dcp in (0, 2):
                                c_pair(st, dcp)
                    else:
                        nc.sync.dma_start(out[0:128, 0:512],
                                          cb16[:, 0:512])
    nc.compile()
    return nc


def host_prep(inputs: dict) -> list:
    """Build per-core input maps (host-side sharding + bf16 relayout)."""
    import ml_dtypes
    bf16 = ml_dtypes.bfloat16

    x = np.asarray(inputs["x"], dtype=np.float32)
    wq = np.asarray(inputs["wq"], dtype=np.float32)
    wk = np.asarray(inputs["wk"], dtype=np.float32)
    wv = np.asarray(inputs["wv"], dtype=np.float32)
    wo = np.asarray(inputs["wo"], dtype=np.float32)
    bq = np.asarray(inputs["bq"], dtype=np.float32)
    bk = np.asarray(inputs["bk"], dtype=np.float32)
    bv = np.asarray(inputs["bv"], dtype=np.float32)

    inv = 1.0 / (10000.0 ** (np.arange(0, DH, 2, dtype=np.float64) / DH))
    ang = np.arange(S, dtype=np.float64)[:, None] * inv[None, :]
    sin = np.repeat(np.sin(ang), 2, axis=1).astype(np.float32)  # [S, DH]
    cos = np.repeat(np.cos(ang), 2, axis=1).astype(np.float32)
    # [NQ, DH, SC]: cosq[qi, p, s] = cos[qi*SC+s, p]
    cosq = np.ascontiguousarray(
        cos.reshape(NQ, SC, DH).transpose(0, 2, 1)).astype(bf16)
    sinq = np.ascontiguousarray(
        sin.reshape(NQ, SC, DH).transpose(0, 2, 1)).astype(bf16)

    P = np.zeros((DH, DH), np.float32)
    idx = np.arange(0, DH, 2)
    P[idx, idx + 1] = -1.0    # out[2i]   = -x[2i+1]
    P[idx + 1, idx] = 1.0     # out[2i+1] =  x[2i]
    PT = np.ascontiguousarray(P.T)

    # packed bf16 consts: PT | ones | mask0..3
    # mask_rr (for diagonal tile with row offset rr): cols < rr*128 -> 0,
    # cols in the rr block -> lower triangle (keep col >= row), rest -> 1.
    tri = (np.arange(128)[None, :] >= np.arange(128)[:, None])
    masks = []
    for rr in range(4):
        m = np.ones((128, SC), np.float32)
        m[:, :rr * 128] = 0.0
        m[:, rr * 128:(rr + 1) * 128] = tri.astype(np.float32)
        masks.append(m)
    cb16 = np.concatenate(
        [PT, np.ones((128, 128), np.float32)] + masks,
        axis=1).astype(bf16)

    # [NQ, 128, NT, SC]: xTq[qi, p, d, s] = x[b][qi*SC+s, d*128+p]
    xTqb = [np.ascontiguousarray(
        x[b].reshape(NQ, SC, NT, 128).transpose(0, 3, 2, 1)).astype(bf16)
        for b in range(B)]

    in_maps = []
    for core in range(N_CORES):
        b, g = divmod(core, G)
        c0 = g * CW
        # [128, NT, CW]: wqb[p, d, c] = wq[c0+c, d*128+p]
        wqb = np.ascontiguousarray(
            wq[c0:c0 + CW, :].reshape(CW, NT, 128).transpose(2, 1, 0)
        ).astype(bf16)
        wkb = np.ascontiguousarray(
            wk[c0:c0 + CW, :].reshape(CW, NT, 128).transpose(2, 1, 0)
        ).astype(bf16)
        wvb = np.ascontiguousarray(
            wv[c0:c0 + CW, :].reshape(CW, NT, 128).transpose(2, 1, 0)
        ).astype(bf16)
        # [128, HPG, D]: wob[p, h, j] = wo[j, c0+h*128+p]
        wob = np.ascontiguousarray(
            wo[:, c0:c0 + CW].reshape(D, HPG, 128).transpose(2, 1, 0)
        ).astype(bf16)
        # packed f32 consts: bvb (broadcast) | bq columns | bk columns
        cf32 = np.zeros((128, CW + 2 * HPG), np.float32)
        cf32[:, 0:CW] = bv[c0:c0 + CW][None, :]
        cf32[:, CW:CW + HPG] = bq[c0:c0 + CW].reshape(HPG, DH).T
        cf32[:, CW + HPG:] = bk[c0:c0 + CW].reshape(HPG, DH).T
        in_maps.append({
            "xTq": xTqb[b],
            "wqb": wqb,
            "wkb": wkb,
            "wvb": wvb,
            "wob": wob,
            "cf32d": cf32,
            "cb16d": cb16,
            "cosq": cosq,
            "sinq": sinq,
        })
    return in_maps


def _get_nc():
    if "nc" not in _NC_CACHE:
        _NC_CACHE["nc"] = build_attn_nc(iters=1)
    return _NC_CACHE["nc"]


def kernel(**inputs) -> np.ndarray:
    from concourse.bass_utils import run_bass_kernel_spmd

    nc = _get_nc()
    in_maps = host_prep(inputs)
    res = run_bass_kernel_spmd(nc, in_maps, core_ids=list(range(N_CORES)))
    bo = np.asarray(inputs["bo"], dtype=np.float32)
    outp = np.zeros((B, S, D), np.float32)
    for core in range(N_CORES):
        outp[core // G] += np.asarray(res.results[core]["out"],
                                      dtype=np.float32)
    outp += bo[None, None, :]
    return outp


# revision 34
# speedup vs baseline: 1.0259x; 1.0259x over previous
"""Trainium2 Bass kernel for 16-head causal self-attention with RoPE.

Problem: x:[2,2048,2048] -> MHA(wq,wk,wv,wo, causal mask, RoPE) -> [2,2048,2048].

Sharding (8 NeuronCores): core = b*4 + g, where b in {0,1} is the batch
(data parallel) and g in {0..3} is a head group of 4 heads (tensor parallel
over the 16 heads / 2048 channels: group g owns channels [g*512, (g+1)*512)).

v3 design (pipelined attention, ACT-instruction minimization):
  - All intermediates SBUF-resident in bf16 as in v2 (q/k RoPE'd [dh,S] per
    head, v [S,cw] tiles, ao [dh, 4*S]); weights/x pre-laid-out on host.
  - Phase B (attention) is restructured around the measured HW costs:
    ACT costs (N+352)/1.2 ns per instruction, so exp is issued once per
    *pair* of score tiles over a 2-bank PSUM tile [128,1024] (80 instead of
    160 activations).  Wide masks [128,512] per diagonal-row-offset zero
    both the causal triangle and the never-written psum garbage, letting the
    PV matmuls run full width and the denominator accumulate with plain
    elementwise adds (alternating DVE/Pool), reduced by one ones-matmul per
    (head, chunk) block.
  - The PE queue is kept busy through exp latency by software pipelining:
    the PV matmuls of score-group g are emitted after the score matmuls +
    exp of group g+1, and independent GEMM work (projection pairs of chunk
    qi+1, out-projection pairs of chunk qi-1) is injected at paced filler
    points between groups.
  - PSUM budget: psA(3 banks: projections, RoPE rotate, out-proj, denom)
    + psS(2x2 banks: score groups) + psO(1 bank: PV accumulator) = 8.
Host: out[b] = sum of the 4 group partials + bo.
"""

import math
import sys

sys.path.insert(0, "/opt/trn_rl_repo")

import numpy as np

N_CORES = 8
B, S, D = 2, 2048, 2048
H, DH = 16, 128
G = 4                 # head groups (tensor-parallel factor per batch)
HPG = H // G          # heads per group = 4
CW = HPG * DH         # channels per group = 512
NT = S // 128         # 16 d-tiles of the contraction dim
SC = 512              # free-dim chunk (one PSUM bank of fp32)
NQ = S // SC          # 4 s-chunks

_NC_CACHE: dict = {}


def build_attn_nc(iters: int = 1, phases: int = 3):
    """Build + compile the Bass module (same program for all 8 cores)."""
    import concourse.tile as tile
    from concourse import bacc, mybir

    f32 = mybir.dt.float32
    bf16 = mybir.dt.bfloat16
    AF = mybir.ActivationFunctionType
    SCALE = 1.0 / math.sqrt(DH)

    nc = bacc.Bacc("TRN2", target_bir_lowering=False, debug=False,
                   num_devices=N_CORES)

    # host-pre-laid-out inputs (see host_prep)
    xTq = nc.dram_tensor("xTq", [NQ, 128, NT, SC], bf16,
                         kind="ExternalInput").ap()
    wqb = nc.dram_tensor("wqb", [128, NT, CW], bf16, kind="ExternalInput").ap()
    wkb = nc.dram_tensor("wkb", [128, NT, CW], bf16, kind="ExternalInput").ap()
    wvb = nc.dram_tensor("wvb", [128, NT, CW], bf16, kind="ExternalInput").ap()
    wob = nc.dram_tensor("wob", [128, HPG, D], bf16, kind="ExternalInput").ap()
    # packed constants: cf32 = bvb | bq | bk, cb16 = PT | ones | mask0..3
    cf32d = nc.dram_tensor("cf32d", [128, CW + 2 * HPG], f32,
                           kind="ExternalInput").ap()
    cb16d = nc.dram_tensor("cb16d", [128, 256 + 4 * SC], bf16,
                           kind="ExternalInput").ap()
    cosq = nc.dram_tensor("cosq", [NQ, DH, SC], bf16,
                          kind="ExternalInput").ap()
    sinq = nc.dram_tensor("sinq", [NQ, DH, SC], bf16,
                          kind="ExternalInput").ap()

    out = nc.dram_tensor("out", [S, D], bf16, kind="ExternalOutput").ap()

    with tile.TileContext(nc) as tc:
        for it in range(iters):
            with tc.tile_pool(name="const", bufs=1) as cpool, \
                 tc.tile_pool(name="wts", bufs=1) as wpool, \
                 tc.tile_pool(name="perst", bufs=1) as ppool:
                # ---- persistent SBUF tensors --------------------------
                qT = [ppool.tile([DH, S], bf16, name=f"qT{h}_{it}",
                                 tag=f"qT{h}") for h in range(HPG)]
                kT = [ppool.tile([DH, S], bf16, name=f"kT{h}_{it}",
                                 tag=f"kT{h}") for h in range(HPG)]
                v_t = [ppool.tile([128, CW], bf16, name=f"v{t}_{it}",
                                  tag=f"v{t}") for t in range(NT)]
                aoT = ppool.tile([128, HPG * S], bf16, name=f"aoT_{it}",
                                 tag="aoT")

                # ---- startup DMAs (order = queue service order) -------
                w_sb = {}
                for nm in ("q", "k"):
                    w_sb[nm] = wpool.tile([128, NT, CW], bf16,
                                          name=f"w{nm}_{it}", tag=f"w{nm}")
                # interleaved ramped parts: first q-pair chain can start
                # ~2us in; wk arrives before the first k-pair chain ends
                for j0, j1 in ((0, 1), (1, 2), (2, 4)):
                    nc.scalar.dma_start(w_sb["q"][:, j0:j1, :],
                                        wqb[:, j0:j1, :])
                nc.scalar.dma_start(w_sb["k"][:, 0:2, :], wkb[:, 0:2, :])
                cb16 = cpool.tile([128, 256 + 4 * SC], bf16,
                                  name=f"cb16{it}", tag="cb16")
                nc.scalar.dma_start(cb16[:], cb16d[:])
                cf32 = cpool.tile([128, CW + 2 * HPG], f32,
                                  name=f"cf32{it}", tag="cf32")
                nc.scalar.dma_start(cf32[:], cf32d[:])
                pt_sb = cb16[:, 0:DH]
                ones_sb = cb16[:, 128:256]
                msk = [cb16[:, 256 + r * SC:256 + (r + 1) * SC]
                       for r in range(4)]
                bvb_sb = cf32[:, 0:CW]
                bq_sb = [cf32[:, CW + ct:CW + ct + 1] for ct in range(HPG)]
                bk_sb = [cf32[:, CW + HPG + ct:CW + HPG + ct + 1]
                         for ct in range(HPG)]
                for j0, j1 in ((4, 8), (8, 16)):
                    nc.scalar.dma_start(w_sb["q"][:, j0:j1, :],
                                        wqb[:, j0:j1, :])
                for j0, j1 in ((2, 4), (4, 8), (8, 16)):
                    nc.scalar.dma_start(w_sb["k"][:, j0:j1, :],
                                        wkb[:, j0:j1, :])
                w_sb["v"] = wpool.tile([128, NT, CW], bf16,
                                       name=f"wv_{it}", tag="wv")
                nc.scalar.dma_start(w_sb["v"][:], wvb[:])
                wo_sb = wpool.tile([128, HPG, D], bf16, name=f"wo{it}",
                                   tag="wo")
                nc.scalar.dma_start(wo_sb[:], wob[:])

                import contextlib
                with contextlib.ExitStack() as est:
                    xqpool = est.enter_context(
                        tc.tile_pool(name="xqp", bufs=2))
                    cspool = est.enter_context(
                        tc.tile_pool(name="csp", bufs=2))
                    prawp = est.enter_context(
                        tc.tile_pool(name="prawp", bufs=4))
                    wkp = est.enter_context(
                        tc.tile_pool(name="workA", bufs=2))
                    atpool = est.enter_context(
                        tc.tile_pool(name="atp", bufs=4))
                    accpool = est.enter_context(
                        tc.tile_pool(name="accp", bufs=2))
                    recpool = est.enter_context(
                        tc.tile_pool(name="recp", bufs=2))
                    outpool = est.enter_context(
                        tc.tile_pool(name="outp", bufs=4))
                    psA = est.enter_context(
                        tc.tile_pool(name="psA", bufs=2, space="PSUM"))
                    psX = est.enter_context(
                        tc.tile_pool(name="psX", bufs=1, space="PSUM"))
                    psS = est.enter_context(
                        tc.tile_pool(name="psS", bufs=2, space="PSUM"))
                    psO = est.enter_context(
                        tc.tile_pool(name="psO", bufs=1, space="PSUM"))

                    def load_chunk(qi):
                        xq = xqpool.tile([128, NT, SC], bf16,
                                         name=f"xq{qi}_{it}", tag="xq")
                        cos_c = cspool.tile([DH, SC], bf16,
                                            name=f"cos{qi}_{it}", tag="cos")
                        sin_c = cspool.tile([DH, SC], bf16,
                                            name=f"sin{qi}_{it}", tag="sin")
                        if qi == 0:
                            # fine ramped parts: first matmul starts early
                            for j0, j1 in ((0, 1), (1, 2), (2, 4), (4, 8),
                                           (8, 16)):
                                nc.sync.dma_start(xq[:, j0:j1, :],
                                                  xTq[qi][:, j0:j1, :])
                        else:
                            nc.sync.dma_start(xq[:], xTq[qi])
                        nc.sync.dma_start(cos_c[:], cosq[qi])
                        nc.sync.dma_start(sin_c[:], sinq[qi])
                        return xq, cos_c, sin_c

                    def a_qk_stream(nm, cp, qi, xq):
                        """Projection pair chain; returns praw tiles for the
                        rope tail (emitted at a later filler point so the PE
                        never waits on the DVE drain)."""
                        bias = bq_sb if nm == "q" else bk_sb
                        psa = psA.tile([128, SC], f32,
                                       name=f"ps{nm}{cp}_{qi}_{it}",
                                       tag="psA")
                        psb = psA.tile([128, SC], f32,
                                       name=f"ps{nm}{cp+1}_{qi}_{it}",
                                       tag="psA")
                        for d in range(NT):
                            nc.tensor.matmul(
                                psa[:],
                                w_sb[nm][:, d, cp * DH:(cp + 1) * DH],
                                xq[:, d, :],
                                start=(d == 0), stop=(d == NT - 1))
                            nc.tensor.matmul(
                                psb[:],
                                w_sb[nm][:, d, (cp + 1) * DH:(cp + 2) * DH],
                                xq[:, d, :],
                                start=(d == 0), stop=(d == NT - 1))
                        praws = []
                        for ct, ps in ((cp, psa), (cp + 1, psb)):
                            praw = prawp.tile([128, SC], bf16,
                                              name=f"praw{nm}{ct}_{qi}_{it}",
                                              tag="praw")
                            nc.vector.tensor_scalar_add(praw[:], ps[:],
                                                        bias[ct])
                            praws.append(praw)
                        return praws

                    def a_qk_rope1(nm, ct, qi, praw, cos_c, sin_c):
                        """Rope tail for ONE ct; emitted at its own filler
                        point so the rotate matmul never waits on DVE."""
                        qkT = qT if nm == "q" else kT
                        psr = psX.tile([128, SC], f32,
                                       name=f"psr{nm}{ct}_{qi}_{it}",
                                       tag="psX")
                        nc.tensor.matmul(psr[:], pt_sb, praw[:],
                                         start=True, stop=True)
                        m1 = wkp.tile([128, SC], bf16,
                                      name=f"m1{nm}{ct}_{qi}_{it}",
                                      tag="m1")
                        nc.vector.tensor_mul(m1[:], praw[:], cos_c[:])
                        m2 = wkp.tile([128, SC], bf16,
                                      name=f"m2{nm}{ct}_{qi}_{it}",
                                      tag="m2")
                        nc.vector.tensor_mul(m2[:], psr[:], sin_c[:])
                        nc.gpsimd.tensor_add(
                            qkT[ct][:, qi * SC:(qi + 1) * SC],
                            m1[:], m2[:])

                    def a_v_pair(sp, qi, xq):
                        psa = psA.tile([128, SC], f32,
                                       name=f"psv{sp}_{qi}_{it}", tag="psA")
                        psb = psA.tile([128, SC], f32,
                                       name=f"psv{sp+1}_{qi}_{it}",
                                       tag="psA")
                        for d in range(NT):
                            nc.tensor.matmul(
                                psa[:],
                                xq[:, d, sp * 128:(sp + 1) * 128],
                                w_sb["v"][:, d, :],
                                start=(d == 0), stop=(d == NT - 1))
                            nc.tensor.matmul(
                                psb[:],
                                xq[:, d, (sp + 1) * 128:(sp + 2) * 128],
                                w_sb["v"][:, d, :],
                                start=(d == 0), stop=(d == NT - 1))
                        nc.vector.tensor_add(v_t[qi * 4 + sp][:], psa[:],
                                             bvb_sb)
                        nc.vector.tensor_add(v_t[qi * 4 + sp + 1][:],
                                             psb[:], bvb_sb)

                    def c_pair(st, dcp):
                        psa = psA.tile([128, SC], f32,
                                       name=f"op{st}{dcp}_{it}", tag="psA")
                        psb = psA.tile([128, SC], f32,
                                       name=f"op{st}{dcp+1}_{it}", tag="psA")
                        for h in range(HPG):
                            lhs = aoT[:, h * S + st * 128:
                                      h * S + (st + 1) * 128]
                            nc.tensor.matmul(
                                psa[:], lhs,
                                wo_sb[:, h, dcp * SC:(dcp + 1) * SC],
                                start=(h == 0), stop=(h == HPG - 1))
                            nc.tensor.matmul(
                                psb[:], lhs,
                                wo_sb[:, h, (dcp + 1) * SC:(dcp + 2) * SC],
                                start=(h == 0), stop=(h == HPG - 1))
                        # copies stay off the ACT queue (strict FIFO, jammed
                        # behind pending exps during phase B)
                        for dc, op in ((dcp, psa), (dcp + 1, psb)):
                            ot = outpool.tile([128, SC], bf16,
                                              name=f"ot{st}{dc}_{it}",
                                              tag="ot")
                            nc.vector.tensor_copy(ot[:], op[:])
                            nc.sync.dma_start(
                                out[st * 128:(st + 1) * 128,
                                    dc * SC:(dc + 1) * SC], ot[:])

                    def b_block(h, c):
                        """Attention for (head h, query chunk c); yields at
                        filler points (once per score group)."""
                        ntile = 4 * c + 4
                        ngrp = ntile // 2
                        oT = psO.tile([DH, SC], f32, name=f"oT{h}{c}_{it}",
                                      tag="oT")
                        acc = accpool.tile([128, SC], bf16,
                                           name=f"acc{h}{c}_{it}", tag="acc")

                        def emit_pv(at, ts):
                            for j, t_ in enumerate(ts):
                                n0 = max(t_ - 4 * c, 0) * 128
                                nc.tensor.matmul(
                                    oT[:, n0:],
                                    v_t[t_][:, h * DH:(h + 1) * DH],
                                    at[:, j * SC + n0:(j + 1) * SC],
                                    start=(t_ == 0), stop=(t_ == ntile - 1),
                                    skip_group_check=True)

                        pend = []
                        for g in range(ngrp):
                            ss = psS.tile([128, 2 * SC], f32,
                                          name=f"ss{h}{c}{g}_{it}", tag="ss")
                            at = atpool.tile([128, 2 * SC], bf16,
                                             name=f"at{h}{c}{g}_{it}",
                                             tag="at")
                            ts = (2 * g, 2 * g + 1)
                            for j, t_ in enumerate(ts):
                                # full width even for diagonal tiles: keeps
                                # every at element a defined finite value
                                # (masked-out cols are zeroed after exp)
                                nc.tensor.matmul(
                                    ss[:, j * SC:(j + 1) * SC],
                                    kT[h][:, t_ * 128:(t_ + 1) * 128],
                                    qT[h][:, c * SC:(c + 1) * SC],
                                    start=True, stop=True)
                            nc.scalar.activation(at[:], ss[:], AF.Exp,
                                                 bias=0.0, scale=SCALE)
                            for j, t_ in enumerate(ts):
                                rr = t_ - 4 * c
                                if rr >= 0:
                                    nc.vector.tensor_mul(
                                        at[:, j * SC:(j + 1) * SC],
                                        at[:, j * SC:(j + 1) * SC],
                                        msk[rr])
                            if g == 0:
                                nc.vector.tensor_add(acc[:], at[:, 0:SC],
                                                     at[:, SC:2 * SC])
                            else:
                                nc.vector.tensor_add(acc[:], acc[:],
                                                     at[:, 0:SC])
                                nc.vector.tensor_add(acc[:], acc[:],
                                                     at[:, SC:2 * SC])
                            pend.append((at, ts))
                            if len(pend) > 2:
                                yield
                                emit_pv(*pend.pop(0))
                            elif g >= 1:
                                yield
                        while pend:
                            yield
                            emit_pv(*pend.pop(0))
                        dnp = psX.tile([128, SC], f32,
                                       name=f"dn{h}{c}_{it}", tag="psX")
                        nc.tensor.matmul(dnp[:], ones_sb, acc[:],
                                         start=True, stop=True)
                        rec = recpool.tile([128, SC], f32,
                                           name=f"rec{h}{c}_{it}", tag="rec")
                        nc.vector.reciprocal(rec[:], dnp[:])
                        nc.vector.tensor_mul(
                            aoT[:, h * S + c * SC:h * S + (c + 1) * SC],
                            oT[:], rec[:])

                    # ---- prologue: A(0), rope tails one stream behind --
                    xq, cos_c, sin_c = load_chunk(0)
                    pr_q0 = a_qk_stream("q", 0, 0, xq)
                    pr_k0 = a_qk_stream("k", 0, 0, xq)
                    a_qk_rope1("q", 0, 0, pr_q0[0], cos_c, sin_c)
                    pr_q2 = a_qk_stream("q", 2, 0, xq)
                    a_qk_rope1("q", 1, 0, pr_q0[1], cos_c, sin_c)
                    a_qk_rope1("k", 0, 0, pr_k0[0], cos_c, sin_c)
                    pr_k2 = a_qk_stream("k", 2, 0, xq)
                    a_qk_rope1("k", 1, 0, pr_k0[1], cos_c, sin_c)
                    a_qk_rope1("q", 2, 0, pr_q2[0], cos_c, sin_c)
                    a_v_pair(0, 0, xq)
                    a_qk_rope1("q", 3, 0, pr_q2[1], cos_c, sin_c)
                    a_qk_rope1("k", 2, 0, pr_k2[0], cos_c, sin_c)
                    a_v_pair(2, 0, xq)
                    a_qk_rope1("k", 3, 0, pr_k2[1], cos_c, sin_c)

                    # ---- main loop: B(qi) + fillers A(qi+1), C(qi-1) --
                    for qi in range(NQ):
                        a_fill, c_fill = [], []
                        if qi + 1 < NQ:
                            xq2, cos2, sin2 = load_chunk(qi + 1)
                            qi1 = qi + 1
                            for nm in ("q", "k"):
                                for cp in (0, 2):
                                    hold = {}

                                    def fs(nm=nm, cp=cp, hold=hold,
                                           qi1=qi1, xq2=xq2):
                                        hold["p"] = a_qk_stream(nm, cp,
                                                                qi1, xq2)

                                    def fr0(nm=nm, cp=cp, hold=hold,
                                            qi1=qi1, cos2=cos2, sin2=sin2):
                                        a_qk_rope1(nm, cp, qi1,
                                                   hold["p"][0], cos2, sin2)

                                    def fr1(nm=nm, cp=cp, hold=hold,
                                            qi1=qi1, cos2=cos2, sin2=sin2):
                                        a_qk_rope1(nm, cp + 1, qi1,
                                                   hold["p"][1], cos2, sin2)

                                    a_fill.append(fs)
                                    a_fill.append(fr0)
                                    a_fill.append(fr1)
                            for sp in (0, 2):
                                a_fill.append(
                                    lambda sp=sp, qi1=qi1, xq2=xq2:
                                    a_v_pair(sp, qi1, xq2))
                        # out-projection fillers: B(1)<-C(0), B(3)<-C(1,2)
                        # (B(2) has plenty of A(3) filler; B(3) has no A)
                        C_SRC = {1: (0,), 3: (1, 2)}
                        if phases >= 3:
                            for cc in C_SRC.get(qi, ()):
                                for st in range(4 * cc, 4 * cc + 4):
                                    for dcp in (0, 2):
                                        c_fill.append(
                                            lambda st=st, dcp=dcp: c_pair(
                                                st, dcp))
                        # interleave A and C fillers
                        fillers = []
                        na, ncf = len(a_fill), len(c_fill)
                        ia = ic = 0
                        for k_ in range(na + ncf):
                            if ia * max(ncf, 1) <= ic * max(na, 1) and \
                                    ia < na:
                                fillers.append(a_fill[ia]); ia += 1
                            elif ic < ncf:
                                fillers.append(c_fill[ic]); ic += 1
                            else:
                                fillers.append(a_fill[ia]); ia += 1

                        if phases >= 2:
                            total_pts = HPG * (2 * qi + 3)
                            done = 0
                            pt = 0
                            for h in range(HPG):
                                for _ in b_block(h, qi):
                                    pt += 1
                                    want = len(fillers) * pt // total_pts
                                    while done < want:
                                        fillers[done]()
                                        done += 1
                            while done < len(fillers):
                                fillers[done]()
                                done += 1
                        else:
                            for fl in fillers:
                                fl()

                    # ---- tail: out-projection for the last chunk ------
                    if phases >= 3:
                        for st in range(4 * (NQ - 1), 4 * NQ):
                            for dcp in (0, 2):
                                c_pair(st, dcp)
                    else:
                        nc.sync.dma_start(out[0:128, 0:512],
                                          cb16[:, 0:512])
    nc.compile()
    return nc


def host_prep(inputs: dict) -> list:
    """Build per-core input maps (host-side sharding + bf16 relayout)."""
    import ml_dtypes
    bf16 = ml_dtypes.bfloat16

    x = np.asarray(inputs["x"], dtype=np.float32)
    wq = np.asarray(inputs["wq"], dtype=np.float32)
    wk = np.asarray(inputs["wk"], dtype=np.float32)
    wv = np.asarray(inputs["wv"], dtype=np.float32)
    wo = np.asarray(inputs["wo"], dtype=np.float32)
    bq = np.asarray(inputs["bq"], dtype=np.float32)
    bk = np.asarray(inputs["bk"], dtype=np.float32)
    bv = np.asarray(inputs["bv"], dtype=np.float32)

    inv = 1.0 / (10000.0 ** (np.arange(0, DH, 2, dtype=np.float64) / DH))
    ang = np.arange(S, dtype=np.float64)[:, None] * inv[None, :]
    sin = np.repeat(np.sin(ang), 2, axis=1).astype(np.float32)  # [S, DH]
    cos = np.repeat(np.cos(ang), 2, axis=1).astype(np.float32)
    # [NQ, DH, SC]: cosq[qi, p, s] = cos[qi*SC+s, p]
    cosq = np.ascontiguousarray(
        cos.reshape(NQ, SC, DH).transpose(0, 2, 1)).astype(bf16)
    sinq = np.ascontiguousarray(
        sin.reshape(NQ, SC, DH).transpose(0, 2, 1)).astype(bf16)

    P = np.zeros((DH, DH), np.float32)
    idx = np.arange(0, DH, 2)
    P[idx, idx + 1] = -1.0    # out[2i]   = -x[2i+1]
    P[idx + 1, idx] = 1.0     # out[2i+1] =  x[2i]
    PT = np.ascontiguousarray(P.T)

    # packed bf16 consts: PT | ones | mask0..3
    # mask_rr (for diagonal tile with row offset rr): cols < rr*128 -> 0,
    # cols in the rr block -> lower triangle (keep col >= row), rest -> 1.
    tri = (np.arange(128)[None, :] >= np.arange(128)[:, None])
    masks = []
    for rr in range(4):
        m = np.ones((128, SC), np.float32)
        m[:, :rr * 128] = 0.0
        m[:, rr * 128:(rr + 1) * 128] = tri.astype(np.float32)
        masks.append(m)
    cb16 = np.concatenate(
        [PT, np.ones((128, 128), np.float32)] + masks,
        axis=1).astype(bf16)

    # [NQ, 128, NT, SC]: xTq[qi, p, d, s] = x[b][qi*SC+s, d*128+p]
    xTqb = [np.ascontiguousarray(
        x[b].reshape(NQ, SC, NT, 128).transpose(0, 3, 2, 1)).astype(bf16)
        for b in range(B)]

    in_maps = []
    for core in range(N_CORES):
        b, g = divmod(core, G)
        c0 = g * CW
        # [128, NT, CW]: wqb[p, d, c] = wq[c0+c, d*128+p]
        wqb = np.ascontiguousarray(
            wq[c0:c0 + CW, :].reshape(CW, NT, 128).transpose(2, 1, 0)
        ).astype(bf16)
        wkb = np.ascontiguousarray(
            wk[c0:c0 + CW, :].reshape(CW, NT, 128).transpose(2, 1, 0)
        ).astype(bf16)
        wvb = np.ascontiguousarray(
            wv[c0:c0 + CW, :].reshape(CW, NT, 128).transpose(2, 1, 0)
        ).astype(bf16)
        # [128, HPG, D]: wob[p, h, j] = wo[j, c0+h*128+p]
        wob = np.ascontiguousarray(
            wo[:, c0:c0 + CW].reshape(D, HPG, 128).transpose(2, 1, 0)
        ).astype(bf16)
        # packed f32 consts: bvb (broadcast) | bq columns | bk columns
        cf32 = np.zeros((128, CW + 2 * HPG), np.float32)
        cf32[:, 0:CW] = bv[c0:c0 + CW][None, :]
        cf32[:, CW:CW + HPG] = bq[c0:c0 + CW].reshape(HPG, DH).T
        cf32[:, CW + HPG:] = bk[c0:c0 + CW].reshape(HPG, DH).T
        in_maps.append({
            "xTq": xTqb[b],
            "wqb": wqb,
            "wkb": wkb,
            "wvb": wvb,
            "wob": wob,
            "cf32d": cf32,
            "cb16d": cb16,
            "cosq": cosq,
            "sinq": sinq,
        })
    return in_maps


def _get_nc():
    if "nc" not in _NC_CACHE:
        _NC_CACHE["nc"] = build_attn_nc(iters=1)
    return _NC_CACHE["nc"]


def kernel(**inputs) -> np.ndarray:
    from concourse.bass_utils import run_bass_kernel_spmd

    nc = _get_nc()
    in_maps = host_prep(inputs)
    res = run_bass_kernel_spmd(nc, in_maps, core_ids=list(range(N_CORES)))
    bo = np.asarray(inputs["bo"], dtype=np.float32)
    outp = np.zeros((B, S, D), np.float32)
    for core in range(N_CORES):
        outp[core // G] += np.asarray(res.results[core]["out"],
                                      dtype=np.float32)
    outp += bo[None, None, :]
    return outp


# revision 40
# speedup vs baseline: 1.0446x; 1.0182x over previous
"""Trainium2 Bass kernel for 16-head causal self-attention with RoPE.

Problem: x:[2,2048,2048] -> MHA(wq,wk,wv,wo, causal mask, RoPE) -> [2,2048,2048].

Sharding (8 NeuronCores): core = b*4 + g, where b in {0,1} is the batch
(data parallel) and g in {0..3} is a head group of 4 heads (tensor parallel
over the 16 heads / 2048 channels: group g owns channels [g*512, (g+1)*512)).

v3 design (pipelined attention, ACT-instruction minimization):
  - All intermediates SBUF-resident in bf16 as in v2 (q/k RoPE'd [dh,S] per
    head, v [S,cw] tiles, ao [dh, 4*S]); weights/x pre-laid-out on host.
  - Phase B (attention) is restructured around the measured HW costs:
    ACT costs (N+352)/1.2 ns per instruction, so exp is issued once per
    *pair* of score tiles over a 2-bank PSUM tile [128,1024] (80 instead of
    160 activations).  Wide masks [128,512] per diagonal-row-offset zero
    both the causal triangle and the never-written psum garbage, letting the
    PV matmuls run full width and the denominator accumulate with plain
    elementwise adds (alternating DVE/Pool), reduced by one ones-matmul per
    (head, chunk) block.
  - The PE queue is kept busy through exp latency by software pipelining:
    the PV matmuls of score-group g are emitted after the score matmuls +
    exp of group g+1, and independent GEMM work (projection pairs of chunk
    qi+1, out-projection pairs of chunk qi-1) is injected at paced filler
    points between groups.
  - PSUM budget: psA(3 banks: projections, RoPE rotate, out-proj, denom)
    + psS(2x2 banks: score groups) + psO(1 bank: PV accumulator) = 8.
Host: out[b] = sum of the 4 group partials + bo.
"""

import math
import sys

sys.path.insert(0, "/opt/trn_rl_repo")

import numpy as np

N_CORES = 8
B, S, D = 2, 2048, 2048
H, DH = 16, 128
G = 4                 # head groups (tensor-parallel factor per batch)
HPG = H // G          # heads per group = 4
CW = HPG * DH         # channels per group = 512
NT = S // 128         # 16 d-tiles of the contraction dim
SC = 512              # free-dim chunk (one PSUM bank of fp32)
NQ = S // SC          # 4 s-chunks

_NC_CACHE: dict = {}


def build_attn_nc(iters: int = 1, phases: int = 3):
    """Build + compile the Bass module (same program for all 8 cores)."""
    import concourse.tile as tile
    from concourse import bacc, mybir

    f32 = mybir.dt.float32
    bf16 = mybir.dt.bfloat16
    AF = mybir.ActivationFunctionType
    SCALE = 1.0 / math.sqrt(DH)

    nc = bacc.Bacc("TRN2", target_bir_lowering=False, debug=False,
                   num_devices=N_CORES)

    # host-pre-laid-out inputs (see host_prep)
    xTq = nc.dram_tensor("xTq", [NQ, 128, NT, SC], bf16,
                         kind="ExternalInput").ap()
    wqb = nc.dram_tensor("wqb", [128, NT, CW], bf16, kind="ExternalInput").ap()
    wkb = nc.dram_tensor("wkb", [128, NT, CW], bf16, kind="ExternalInput").ap()
    wvb = nc.dram_tensor("wvb", [128, NT, CW], bf16, kind="ExternalInput").ap()
    wob = nc.dram_tensor("wob", [128, HPG, D], bf16, kind="ExternalInput").ap()
    # packed constants: cf32 = bvb | bq | bk, cb16 = PT | ones | mask0..3
    cf32d = nc.dram_tensor("cf32d", [128, CW + 2 * HPG], f32,
                           kind="ExternalInput").ap()
    cb16d = nc.dram_tensor("cb16d", [128, 256 + 4 * SC], bf16,
                           kind="ExternalInput").ap()
    cosq = nc.dram_tensor("cosq", [NQ, DH, SC], bf16,
                          kind="ExternalInput").ap()
    sinq = nc.dram_tensor("sinq", [NQ, DH, SC], bf16,
                          kind="ExternalInput").ap()

    out = nc.dram_tensor("out", [S, D], bf16, kind="ExternalOutput").ap()

    with tile.TileContext(nc) as tc:
        for it in range(iters):
            with tc.tile_pool(name="const", bufs=1) as cpool, \
                 tc.tile_pool(name="wts", bufs=1) as wpool, \
                 tc.tile_pool(name="perst", bufs=1) as ppool:
                # ---- persistent SBUF tensors --------------------------
                qT = [ppool.tile([DH, S], bf16, name=f"qT{h}_{it}",
                                 tag=f"qT{h}") for h in range(HPG)]
                kT = [ppool.tile([DH, S], bf16, name=f"kT{h}_{it}",
                                 tag=f"kT{h}") for h in range(HPG)]
                v_t = [ppool.tile([128, CW], bf16, name=f"v{t}_{it}",
                                  tag=f"v{t}") for t in range(NT)]
                aoT = ppool.tile([128, HPG * S], bf16, name=f"aoT_{it}",
                                 tag="aoT")

                # ---- startup DMAs (order = queue service order) -------
                w_sb = {}
                for nm in ("q", "k"):
                    w_sb[nm] = wpool.tile([128, NT, CW], bf16,
                                          name=f"w{nm}_{it}", tag=f"w{nm}")
                # interleaved ramped parts: first q-pair chain can start
                # ~2us in; wk arrives before the first k-pair chain ends
                for j0, j1 in ((0, 1), (1, 2), (2, 4)):
                    nc.scalar.dma_start(w_sb["q"][:, j0:j1, :],
                                        wqb[:, j0:j1, :])
                nc.scalar.dma_start(w_sb["k"][:, 0:2, :], wkb[:, 0:2, :])
                cb16 = cpool.tile([128, 256 + 4 * SC], bf16,
                                  name=f"cb16{it}", tag="cb16")
                nc.scalar.dma_start(cb16[:], cb16d[:])
                cf32 = cpool.tile([128, CW + 2 * HPG], f32,
                                  name=f"cf32{it}", tag="cf32")
                nc.scalar.dma_start(cf32[:], cf32d[:])
                pt_sb = cb16[:, 0:DH]
                ones_sb = cb16[:, 128:256]
                msk = [cb16[:, 256 + r * SC:256 + (r + 1) * SC]
                       for r in range(4)]
                bvb_sb = cf32[:, 0:CW]
                bq_sb = [cf32[:, CW + ct:CW + ct + 1] for ct in range(HPG)]
                bk_sb = [cf32[:, CW + HPG + ct:CW + HPG + ct + 1]
                         for ct in range(HPG)]
                for j0, j1 in ((4, 8), (8, 16)):
                    nc.scalar.dma_start(w_sb["q"][:, j0:j1, :],
                                        wqb[:, j0:j1, :])
                for j0, j1 in ((2, 4), (4, 8), (8, 16)):
                    nc.scalar.dma_start(w_sb["k"][:, j0:j1, :],
                                        wkb[:, j0:j1, :])
                w_sb["v"] = wpool.tile([128, NT, CW], bf16,
                                       name=f"wv_{it}", tag="wv")
                nc.scalar.dma_start(w_sb["v"][:], wvb[:])
                wo_sb = wpool.tile([128, HPG, D], bf16, name=f"wo{it}",
                                   tag="wo")
                nc.scalar.dma_start(wo_sb[:], wob[:])

                import contextlib
                with contextlib.ExitStack() as est:
                    xqpool = est.enter_context(
                        tc.tile_pool(name="xqp", bufs=2))
                    cspool = est.enter_context(
                        tc.tile_pool(name="csp", bufs=2))
                    prawp = est.enter_context(
                        tc.tile_pool(name="prawp", bufs=4))
                    wkp = est.enter_context(
                        tc.tile_pool(name="workA", bufs=2))
                    atpool = est.enter_context(
                        tc.tile_pool(name="atp", bufs=4))
                    accpool = est.enter_context(
                        tc.tile_pool(name="accp", bufs=2))
                    recpool = est.enter_context(
                        tc.tile_pool(name="recp", bufs=2))
                    outpool = est.enter_context(
                        tc.tile_pool(name="outp", bufs=4))
                    psA = est.enter_context(
                        tc.tile_pool(name="psA", bufs=3, space="PSUM"))
                    psS = est.enter_context(
                        tc.tile_pool(name="psS", bufs=2, space="PSUM"))
                    psO = est.enter_context(
                        tc.tile_pool(name="psO", bufs=1, space="PSUM"))

                    def load_chunk(qi):
                        xq = xqpool.tile([128, NT, SC], bf16,
                                         name=f"xq{qi}_{it}", tag="xq")
                        cos_c = cspool.tile([DH, SC], bf16,
                                            name=f"cos{qi}_{it}", tag="cos")
                        sin_c = cspool.tile([DH, SC], bf16,
                                            name=f"sin{qi}_{it}", tag="sin")
                        if qi == 0:
                            # fine ramped parts: first matmul starts early
                            for j0, j1 in ((0, 1), (1, 2), (2, 4), (4, 8),
                                           (8, 16)):
                                nc.sync.dma_start(xq[:, j0:j1, :],
                                                  xTq[qi][:, j0:j1, :])
                        else:
                            nc.sync.dma_start(xq[:], xTq[qi])
                        nc.sync.dma_start(cos_c[:], cosq[qi])
                        nc.sync.dma_start(sin_c[:], sinq[qi])
                        return xq, cos_c, sin_c

                    def a_qk_pair(nm, cp, qi, xq, cos_c, sin_c):
                        qkT = qT if nm == "q" else kT
                        bias = bq_sb if nm == "q" else bk_sb
                        psa = psA.tile([128, SC], f32,
                                       name=f"ps{nm}{cp}_{qi}_{it}",
                                       tag="psA")
                        psb = psA.tile([128, SC], f32,
                                       name=f"ps{nm}{cp+1}_{qi}_{it}",
                                       tag="psA")
                        for d in range(NT):
                            nc.tensor.matmul(
                                psa[:],
                                w_sb[nm][:, d, cp * DH:(cp + 1) * DH],
                                xq[:, d, :],
                                start=(d == 0), stop=(d == NT - 1))
                            nc.tensor.matmul(
                                psb[:],
                                w_sb[nm][:, d, (cp + 1) * DH:(cp + 2) * DH],
                                xq[:, d, :],
                                start=(d == 0), stop=(d == NT - 1))
                        for ct, ps in ((cp, psa), (cp + 1, psb)):
                            praw = prawp.tile([128, SC], bf16,
                                              name=f"praw{nm}{ct}_{qi}_{it}",
                                              tag="praw")
                            nc.vector.tensor_scalar_add(praw[:], ps[:],
                                                        bias[ct])
                            psr = psA.tile([128, SC], f32,
                                           name=f"psr{nm}{ct}_{qi}_{it}",
                                           tag="psA")
                            nc.tensor.matmul(psr[:], pt_sb, praw[:],
                                             start=True, stop=True)
                            m1 = wkp.tile([128, SC], bf16,
                                          name=f"m1{nm}{ct}_{qi}_{it}",
                                          tag="m1")
                            nc.vector.tensor_mul(m1[:], praw[:], cos_c[:])
                            m2 = wkp.tile([128, SC], bf16,
                                          name=f"m2{nm}{ct}_{qi}_{it}",
                                          tag="m2")
                            nc.vector.tensor_mul(m2[:], psr[:], sin_c[:])
                            nc.gpsimd.tensor_add(
                                qkT[ct][:, qi * SC:(qi + 1) * SC],
                                m1[:], m2[:])

                    def a_v_pair(sp, qi, xq):
                        psa = psA.tile([128, SC], f32,
                                       name=f"psv{sp}_{qi}_{it}", tag="psA")
                        psb = psA.tile([128, SC], f32,
                                       name=f"psv{sp+1}_{qi}_{it}",
                                       tag="psA")
                        for d in range(NT):
                            nc.tensor.matmul(
                                psa[:],
                                xq[:, d, sp * 128:(sp + 1) * 128],
                                w_sb["v"][:, d, :],
                                start=(d == 0), stop=(d == NT - 1))
                            nc.tensor.matmul(
                                psb[:],
                                xq[:, d, (sp + 1) * 128:(sp + 2) * 128],
                                w_sb["v"][:, d, :],
                                start=(d == 0), stop=(d == NT - 1))
                        nc.vector.tensor_add(v_t[qi * 4 + sp][:], psa[:],
                                             bvb_sb)
                        nc.vector.tensor_add(v_t[qi * 4 + sp + 1][:],
                                             psb[:], bvb_sb)

                    def c_pair(st, dcp):
                        psa = psA.tile([128, SC], f32,
                                       name=f"op{st}{dcp}_{it}", tag="psA")
                        psb = psA.tile([128, SC], f32,
                                       name=f"op{st}{dcp+1}_{it}", tag="psA")
                        for h in range(HPG):
                            lhs = aoT[:, h * S + st * 128:
                                      h * S + (st + 1) * 128]
                            nc.tensor.matmul(
                                psa[:], lhs,
                                wo_sb[:, h, dcp * SC:(dcp + 1) * SC],
                                start=(h == 0), stop=(h == HPG - 1))
                            nc.tensor.matmul(
                                psb[:], lhs,
                                wo_sb[:, h, (dcp + 1) * SC:(dcp + 2) * SC],
                                start=(h == 0), stop=(h == HPG - 1))
                        for dc, op in ((dcp, psa), (dcp + 1, psb)):
                            ot = outpool.tile([128, SC], bf16,
                                              name=f"ot{st}{dc}_{it}",
                                              tag="ot")
                            if dc % 2 == 0:
                                nc.vector.tensor_copy(ot[:], op[:])
                            else:
                                nc.scalar.activation(ot[:], op[:], AF.Copy)
                            nc.sync.dma_start(
                                out[st * 128:(st + 1) * 128,
                                    dc * SC:(dc + 1) * SC], ot[:])

                    def b_block(h, c):
                        """Attention for (head h, query chunk c); yields at
                        filler points (once per score group)."""
                        ntile = 4 * c + 4
                        ngrp = ntile // 2
                        oT = psO.tile([DH, SC], f32, name=f"oT{h}{c}_{it}",
                                      tag="oT")
                        acc = accpool.tile([128, SC], bf16,
                                           name=f"acc{h}{c}_{it}", tag="acc")

                        def emit_pv(at, ts):
                            for j, t_ in enumerate(ts):
                                n0 = max(t_ - 4 * c, 0) * 128
                                nc.tensor.matmul(
                                    oT[:, n0:],
                                    v_t[t_][:, h * DH:(h + 1) * DH],
                                    at[:, j * SC + n0:(j + 1) * SC],
                                    start=(t_ == 0), stop=(t_ == ntile - 1),
                                    skip_group_check=True)

                        pend = []
                        for g in range(ngrp):
                            ss = psS.tile([128, 2 * SC], f32,
                                          name=f"ss{h}{c}{g}_{it}", tag="ss")
                            at = atpool.tile([128, 2 * SC], bf16,
                                             name=f"at{h}{c}{g}_{it}",
                                             tag="at")
                            ts = (2 * g, 2 * g + 1)
                            for j, t_ in enumerate(ts):
                                # full width even for diagonal tiles: keeps
                                # every at element a defined finite value
                                # (masked-out cols are zeroed after exp)
                                nc.tensor.matmul(
                                    ss[:, j * SC:(j + 1) * SC],
                                    kT[h][:, t_ * 128:(t_ + 1) * 128],
                                    qT[h][:, c * SC:(c + 1) * SC],
                                    start=True, stop=True)
                            nc.scalar.activation(at[:], ss[:], AF.Exp,
                                                 bias=0.0, scale=SCALE)
                            for j, t_ in enumerate(ts):
                                rr = t_ - 4 * c
                                if rr >= 0:
                                    nc.vector.tensor_mul(
                                        at[:, j * SC:(j + 1) * SC],
                                        at[:, j * SC:(j + 1) * SC],
                                        msk[rr])
                            if g == 0:
                                nc.vector.tensor_add(acc[:], at[:, 0:SC],
                                                     at[:, SC:2 * SC])
                            else:
                                nc.vector.tensor_add(acc[:], acc[:],
                                                     at[:, 0:SC])
                                nc.vector.tensor_add(acc[:], acc[:],
                                                     at[:, SC:2 * SC])
                            pend.append((at, ts))
                            if len(pend) > 2:
                                yield
                                emit_pv(*pend.pop(0))
                            elif g >= 1:
                                yield
                        while pend:
                            yield
                            emit_pv(*pend.pop(0))
                        dnp = psA.tile([128, SC], f32,
                                       name=f"dn{h}{c}_{it}", tag="psA")
                        nc.tensor.matmul(dnp[:], ones_sb, acc[:],
                                         start=True, stop=True)
                        rec = recpool.tile([128, SC], f32,
                                           name=f"rec{h}{c}_{it}", tag="rec")
                        nc.vector.reciprocal(rec[:], dnp[:])
                        nc.vector.tensor_mul(
                            aoT[:, h * S + c * SC:h * S + (c + 1) * SC],
                            oT[:], rec[:])

                    # ---- prologue: A(0) -------------------------------
                    xq, cos_c, sin_c = load_chunk(0)
                    for cp in (0, 2):
                        a_qk_pair("q", cp, 0, xq, cos_c, sin_c)
                        a_qk_pair("k", cp, 0, xq, cos_c, sin_c)
                    for sp in (0, 2):
                        a_v_pair(sp, 0, xq)

                    # ---- main loop: B(qi) + fillers A(qi+1), C(qi-1) --
                    for qi in range(NQ):
                        a_fill, c_fill = [], []
                        if qi + 1 < NQ:
                            xq2, cos2, sin2 = load_chunk(qi + 1)
                            qi1 = qi + 1
                            for nm in ("q", "k"):
                                for cp in (0, 2):
                                    a_fill.append(
                                        lambda nm=nm, cp=cp, qi1=qi1,
                                        xq2=xq2, cos2=cos2, sin2=sin2:
                                        a_qk_pair(nm, cp, qi1, xq2,
                                                  cos2, sin2))
                            for sp in (0, 2):
                                a_fill.append(
                                    lambda sp=sp, qi1=qi1, xq2=xq2:
                                    a_v_pair(sp, qi1, xq2))
                        # out-projection fillers: B(1)<-C(0), B(3)<-C(1,2)
                        # (B(2) has plenty of A(3) filler; B(3) has no A)
                        C_SRC = {1: (0,), 3: (1, 2)}
                        if phases >= 3:
                            for cc in C_SRC.get(qi, ()):
                                for st in range(4 * cc, 4 * cc + 4):
                                    for dcp in (0, 2):
                                        c_fill.append(
                                            lambda st=st, dcp=dcp: c_pair(
                                                st, dcp))
                        # interleave A and C fillers
                        fillers = []
                        na, ncf = len(a_fill), len(c_fill)
                        ia = ic = 0
                        for k_ in range(na + ncf):
                            if ia * max(ncf, 1) <= ic * max(na, 1) and \
                                    ia < na:
                                fillers.append(a_fill[ia]); ia += 1
                            elif ic < ncf:
                                fillers.append(c_fill[ic]); ic += 1
                            else:
                                fillers.append(a_fill[ia]); ia += 1

                        if phases >= 2:
                            total_pts = HPG * (2 * qi + 3)
                            done = 0
                            pt = 0
                            for h in range(HPG):
                                for _ in b_block(h, qi):
                                    pt += 1
                                    want = len(fillers) * pt // total_pts
                                    while done < want:
                                        fillers[done]()
                                        done += 1
                            while done < len(fillers):
                                fillers[done]()
                                done += 1
                        else:
                            for fl in fillers:
                                fl()

                    # ---- tail: out-projection for the last chunk ------
                    if phases >= 3:
                        for st in range(4 * (NQ - 1), 4 * NQ):
                            for dcp in (0, 2):
                                c_pair(st, dcp)
                    else:
                        nc.sync.dma_start(out[0:128, 0:512],
                                          cb16[:, 0:512])
    nc.compile()
    return nc


def host_prep(inputs: dict) -> list:
    """Build per-core input maps (host-side sharding + bf16 relayout)."""
    import ml_dtypes
    bf16 = ml_dtypes.bfloat16

    x = np.asarray(inputs["x"], dtype=np.float32)
    wq = np.asarray(inputs["wq"], dtype=np.float32)
    wk = np.asarray(inputs["wk"], dtype=np.float32)
    wv = np.asarray(inputs["wv"], dtype=np.float32)
    wo = np.asarray(inputs["wo"], dtype=np.float32)
    bq = np.asarray(inputs["bq"], dtype=np.float32)
    bk = np.asarray(inputs["bk"], dtype=np.float32)
    bv = np.asarray(inputs["bv"], dtype=np.float32)

    inv = 1.0 / (10000.0 ** (np.arange(0, DH, 2, dtype=np.float64) / DH))
    ang = np.arange(S, dtype=np.float64)[:, None] * inv[None, :]
    sin = np.repeat(np.sin(ang), 2, axis=1).astype(np.float32)  # [S, DH]
    cos = np.repeat(np.cos(ang), 2, axis=1).astype(np.float32)
    # [NQ, DH, SC]: cosq[qi, p, s] = cos[qi*SC+s, p]
    cosq = np.ascontiguousarray(
        cos.reshape(NQ, SC, DH).transpose(0, 2, 1)).astype(bf16)
    sinq = np.ascontiguousarray(
        sin.reshape(NQ, SC, DH).transpose(0, 2, 1)).astype(bf16)

    P = np.zeros((DH, DH), np.float32)
    idx = np.arange(0, DH, 2)
    P[idx, idx + 1] = -1.0    # out[2i]   = -x[2i+1]
    P[idx + 1, idx] = 1.0     # out[2i+1] =  x[2i]
    PT = np.ascontiguousarray(P.T)

    # packed bf16 consts: PT | ones | mask0..3
    # mask_rr (for diagonal tile with row offset rr): cols < rr*128 -> 0,
    # cols in the rr block -> lower triangle (keep col >= row), rest -> 1.
    tri = (np.arange(128)[None, :] >= np.arange(128)[:, None])
    masks = []
    for rr in range(4):
        m = np.ones((128, SC), np.float32)
        m[:, :rr * 128] = 0.0
        m[:, rr * 128:(rr + 1) * 128] = tri.astype(np.float32)
        masks.append(m)
    cb16 = np.concatenate(
        [PT, np.ones((128, 128), np.float32)] + masks,
        axis=1).astype(bf16)

    # [NQ, 128, NT, SC]: xTq[qi, p, d, s] = x[b][qi*SC+s, d*128+p]
    xTqb = [np.ascontiguousarray(
        x[b].reshape(NQ, SC, NT, 128).transpose(0, 3, 2, 1)).astype(bf16)
        for b in range(B)]

    in_maps = []
    for core in range(N_CORES):
        b, g = divmod(core, G)
        c0 = g * CW
        # [128, NT, CW]: wqb[p, d, c] = wq[c0+c, d*128+p]
        wqb = np.ascontiguousarray(
            wq[c0:c0 + CW, :].reshape(CW, NT, 128).transpose(2, 1, 0)
        ).astype(bf16)
        wkb = np.ascontiguousarray(
            wk[c0:c0 + CW, :].reshape(CW, NT, 128).transpose(2, 1, 0)
        ).astype(bf16)
        wvb = np.ascontiguousarray(
            wv[c0:c0 + CW, :].reshape(CW, NT, 128).transpose(2, 1, 0)
        ).astype(bf16)
        # [128, HPG, D]: wob[p, h, j] = wo[j, c0+h*128+p]
        wob = np.ascontiguousarray(
            wo[:, c0:c0 + CW].reshape(D, HPG, 128).transpose(2, 1, 0)
        ).astype(bf16)
        # packed f32 consts: bvb (broadcast) | bq columns | bk columns
        cf32 = np.zeros((128, CW + 2 * HPG), np.float32)
        cf32[:, 0:CW] = bv[c0:c0 + CW][None, :]
        cf32[:, CW:CW + HPG] = bq[c0:c0 + CW].reshape(HPG, DH).T
        cf32[:, CW + HPG:] = bk[c0:c0 + CW].reshape(HPG, DH).T
        in_maps.append({
            "xTq": xTqb[b],
            "wqb": wqb,
            "wkb": wkb,
            "wvb": wvb,
            "wob": wob,
            "cf32d": cf32,
            "cb16d": cb16,
            "cosq": cosq,
            "sinq": sinq,
        })
    return in_maps


def _get_nc():
    if "nc" not in _NC_CACHE:
        _NC_CACHE["nc"] = build_attn_nc(iters=1)
    return _NC_CACHE["nc"]


def kernel(**inputs) -> np.ndarray:
    from concourse.bass_utils import run_bass_kernel_spmd

    nc = _get_nc()
    in_maps = host_prep(inputs)
    res = run_bass_kernel_spmd(nc, in_maps, core_ids=list(range(N_CORES)))
    bo = np.asarray(inputs["bo"], dtype=np.float32)
    outp = np.zeros((B, S, D), np.float32)
    for core in range(N_CORES):
        outp[core // G] += np.asarray(res.results[core]["out"],
                                      dtype=np.float32)
    outp += bo[None, None, :]
    return outp


# revision 43
# speedup vs baseline: 1.0722x; 1.0264x over previous
"""Trainium2 Bass kernel for 16-head causal self-attention with RoPE.

Problem: x:[2,2048,2048] -> MHA(wq,wk,wv,wo, causal mask, RoPE) -> [2,2048,2048].

Sharding (8 NeuronCores): core = b*4 + g, where b in {0,1} is the batch
(data parallel) and g in {0..3} is a head group of 4 heads (tensor parallel
over the 16 heads / 2048 channels: group g owns channels [g*512, (g+1)*512)).

v3 design (pipelined attention, ACT-instruction minimization):
  - All intermediates SBUF-resident in bf16 as in v2 (q/k RoPE'd [dh,S] per
    head, v [S,cw] tiles, ao [dh, 4*S]); weights/x pre-laid-out on host.
  - Phase B (attention) is restructured around the measured HW costs:
    ACT costs (N+352)/1.2 ns per instruction, so exp is issued once per
    *pair* of score tiles over a 2-bank PSUM tile [128,1024] (80 instead of
    160 activations).  Wide masks [128,512] per diagonal-row-offset zero
    both the causal triangle and the never-written psum garbage, letting the
    PV matmuls run full width and the denominator accumulate with plain
    elementwise adds (alternating DVE/Pool), reduced by one ones-matmul per
    (head, chunk) block.
  - The PE queue is kept busy through exp latency by software pipelining:
    the PV matmuls of score-group g are emitted after the score matmuls +
    exp of group g+1, and independent GEMM work (projection pairs of chunk
    qi+1, out-projection pairs of chunk qi-1) is injected at paced filler
    points between groups.
  - PSUM budget: psA(3 banks: projections, RoPE rotate, out-proj, denom)
    + psS(2x2 banks: score groups) + psO(1 bank: PV accumulator) = 8.
Host: out[b] = sum of the 4 group partials + bo.
"""

import math
import sys

sys.path.insert(0, "/opt/trn_rl_repo")

import numpy as np

N_CORES = 8
B, S, D = 2, 2048, 2048
H, DH = 16, 128
G = 4                 # head groups (tensor-parallel factor per batch)
HPG = H // G          # heads per group = 4
CW = HPG * DH         # channels per group = 512
NT = S // 128         # 16 d-tiles of the contraction dim
SC = 512              # free-dim chunk (one PSUM bank of fp32)
NQ = S // SC          # 4 s-chunks

_NC_CACHE: dict = {}


def build_attn_nc(iters: int = 1, phases: int = 3):
    """Build + compile the Bass module (same program for all 8 cores)."""
    import concourse.tile as tile
    from concourse import bacc, mybir

    f32 = mybir.dt.float32
    bf16 = mybir.dt.bfloat16
    AF = mybir.ActivationFunctionType
    SCALE = 1.0 / math.sqrt(DH)

    nc = bacc.Bacc("TRN2", target_bir_lowering=False, debug=False,
                   num_devices=N_CORES)

    # host-pre-laid-out inputs (see host_prep)
    xTq = nc.dram_tensor("xTq", [NQ, 128, NT, SC], bf16,
                         kind="ExternalInput").ap()
    wqb = nc.dram_tensor("wqb", [128, NT, CW], bf16, kind="ExternalInput").ap()
    wkb = nc.dram_tensor("wkb", [128, NT, CW], bf16, kind="ExternalInput").ap()
    wvb = nc.dram_tensor("wvb", [128, NT, CW], bf16, kind="ExternalInput").ap()
    wob = nc.dram_tensor("wob", [128, HPG, D], bf16, kind="ExternalInput").ap()
    # packed constants: cf32 = bvb | bq | bk, cb16 = PT | ones | mask0..3
    cf32d = nc.dram_tensor("cf32d", [128, CW + 2 * HPG], f32,
                           kind="ExternalInput").ap()
    cb16d = nc.dram_tensor("cb16d", [128, 256 + 4 * SC], bf16,
                           kind="ExternalInput").ap()
    cosq = nc.dram_tensor("cosq", [NQ, DH, SC], bf16,
                          kind="ExternalInput").ap()
    sinq = nc.dram_tensor("sinq", [NQ, DH, SC], bf16,
                          kind="ExternalInput").ap()

    out = nc.dram_tensor("out", [S, D], bf16, kind="ExternalOutput").ap()

    with tile.TileContext(nc) as tc:
        for it in range(iters):
            with tc.tile_pool(name="const", bufs=1) as cpool, \
                 tc.tile_pool(name="wts", bufs=1) as wpool, \
                 tc.tile_pool(name="perst", bufs=1) as ppool:
                # ---- persistent SBUF tensors --------------------------
                qT = [ppool.tile([DH, S], bf16, name=f"qT{h}_{it}",
                                 tag=f"qT{h}") for h in range(HPG)]
                kT = [ppool.tile([DH, S], bf16, name=f"kT{h}_{it}",
                                 tag=f"kT{h}") for h in range(HPG)]
                v_t = [ppool.tile([128, CW], bf16, name=f"v{t}_{it}",
                                  tag=f"v{t}") for t in range(NT)]
                aoT = ppool.tile([128, HPG * S], bf16, name=f"aoT_{it}",
                                 tag="aoT")

                # ---- startup DMAs (order = queue service order) -------
                w_sb = {}
                for nm in ("q", "k"):
                    w_sb[nm] = wpool.tile([128, NT, CW], bf16,
                                          name=f"w{nm}_{it}", tag=f"w{nm}")
                # interleaved ramped parts: first q-pair chain can start
                # ~2us in; wk arrives before the first k-pair chain ends
                for j0, j1 in ((0, 1), (1, 2), (2, 4)):
                    nc.scalar.dma_start(w_sb["q"][:, j0:j1, :],
                                        wqb[:, j0:j1, :])
                nc.scalar.dma_start(w_sb["k"][:, 0:2, :], wkb[:, 0:2, :])
                cb16 = cpool.tile([128, 256 + 4 * SC], bf16,
                                  name=f"cb16{it}", tag="cb16")
                nc.scalar.dma_start(cb16[:], cb16d[:])
                cf32 = cpool.tile([128, CW + 2 * HPG], f32,
                                  name=f"cf32{it}", tag="cf32")
                nc.scalar.dma_start(cf32[:], cf32d[:])
                pt_sb = cb16[:, 0:DH]
                ones_sb = cb16[:, 128:256]
                msk = [cb16[:, 256 + r * SC:256 + (r + 1) * SC]
                       for r in range(4)]
                bvb_sb = cf32[:, 0:CW]
                bq_sb = [cf32[:, CW + ct:CW + ct + 1] for ct in range(HPG)]
                bk_sb = [cf32[:, CW + HPG + ct:CW + HPG + ct + 1]
                         for ct in range(HPG)]
                for j0, j1 in ((4, 8), (8, 16)):
                    nc.scalar.dma_start(w_sb["q"][:, j0:j1, :],
                                        wqb[:, j0:j1, :])
                for j0, j1 in ((2, 4), (4, 8), (8, 16)):
                    nc.scalar.dma_start(w_sb["k"][:, j0:j1, :],
                                        wkb[:, j0:j1, :])
                w_sb["v"] = wpool.tile([128, NT, CW], bf16,
                                       name=f"wv_{it}", tag="wv")
                nc.scalar.dma_start(w_sb["v"][:], wvb[:])
                wo_sb = wpool.tile([128, HPG, D], bf16, name=f"wo{it}",
                                   tag="wo")
                nc.scalar.dma_start(wo_sb[:], wob[:])

                import contextlib
                with contextlib.ExitStack() as est:
                    xqpool = est.enter_context(
                        tc.tile_pool(name="xqp", bufs=2))
                    cspool = est.enter_context(
                        tc.tile_pool(name="csp", bufs=2))
                    prawp = est.enter_context(
                        tc.tile_pool(name="prawp", bufs=4))
                    wkp = est.enter_context(
                        tc.tile_pool(name="workA", bufs=2))
                    atpool = est.enter_context(
                        tc.tile_pool(name="atp", bufs=5))
                    accpool = est.enter_context(
                        tc.tile_pool(name="accp", bufs=2))
                    recpool = est.enter_context(
                        tc.tile_pool(name="recp", bufs=2))
                    outpool = est.enter_context(
                        tc.tile_pool(name="outp", bufs=4))
                    psA = est.enter_context(
                        tc.tile_pool(name="psA", bufs=3, space="PSUM"))
                    psS = est.enter_context(
                        tc.tile_pool(name="psS", bufs=2, space="PSUM"))
                    psO = est.enter_context(
                        tc.tile_pool(name="psO", bufs=1, space="PSUM"))

                    def load_chunk(qi):
                        xq = xqpool.tile([128, NT, SC], bf16,
                                         name=f"xq{qi}_{it}", tag="xq")
                        cos_c = cspool.tile([DH, SC], bf16,
                                            name=f"cos{qi}_{it}", tag="cos")
                        sin_c = cspool.tile([DH, SC], bf16,
                                            name=f"sin{qi}_{it}", tag="sin")
                        if qi == 0:
                            # fine ramped parts: first matmul starts early
                            for j0, j1 in ((0, 1), (1, 2), (2, 4), (4, 8),
                                           (8, 16)):
                                nc.sync.dma_start(xq[:, j0:j1, :],
                                                  xTq[qi][:, j0:j1, :])
                        else:
                            nc.sync.dma_start(xq[:], xTq[qi])
                        nc.sync.dma_start(cos_c[:], cosq[qi])
                        nc.sync.dma_start(sin_c[:], sinq[qi])
                        return xq, cos_c, sin_c

                    def a_qk_pair(nm, cp, qi, xq, cos_c, sin_c):
                        qkT = qT if nm == "q" else kT
                        bias = bq_sb if nm == "q" else bk_sb
                        psa = psA.tile([128, SC], f32,
                                       name=f"ps{nm}{cp}_{qi}_{it}",
                                       tag="psA")
                        psb = psA.tile([128, SC], f32,
                                       name=f"ps{nm}{cp+1}_{qi}_{it}",
                                       tag="psA")
                        for d in range(NT):
                            nc.tensor.matmul(
                                psa[:],
                                w_sb[nm][:, d, cp * DH:(cp + 1) * DH],
                                xq[:, d, :],
                                start=(d == 0), stop=(d == NT - 1))
                            nc.tensor.matmul(
                                psb[:],
                                w_sb[nm][:, d, (cp + 1) * DH:(cp + 2) * DH],
                                xq[:, d, :],
                                start=(d == 0), stop=(d == NT - 1))
                        for ct, ps in ((cp, psa), (cp + 1, psb)):
                            praw = prawp.tile([128, SC], bf16,
                                              name=f"praw{nm}{ct}_{qi}_{it}",
                                              tag="praw")
                            nc.vector.tensor_scalar_add(praw[:], ps[:],
                                                        bias[ct])
                            psr = psA.tile([128, SC], f32,
                                           name=f"psr{nm}{ct}_{qi}_{it}",
                                           tag="psA")
                            nc.tensor.matmul(psr[:], pt_sb, praw[:],
                                             start=True, stop=True)
                            m1 = wkp.tile([128, SC], bf16,
                                          name=f"m1{nm}{ct}_{qi}_{it}",
                                          tag="m1")
                            nc.vector.tensor_mul(m1[:], praw[:], cos_c[:])
                            m2 = wkp.tile([128, SC], bf16,
                                          name=f"m2{nm}{ct}_{qi}_{it}",
                                          tag="m2")
                            nc.vector.tensor_mul(m2[:], psr[:], sin_c[:])
                            nc.gpsimd.tensor_add(
                                qkT[ct][:, qi * SC:(qi + 1) * SC],
                                m1[:], m2[:])

                    def a_v_pair(sp, qi, xq):
                        psa = psA.tile([128, SC], f32,
                                       name=f"psv{sp}_{qi}_{it}", tag="psA")
                        psb = psA.tile([128, SC], f32,
                                       name=f"psv{sp+1}_{qi}_{it}",
                                       tag="psA")
                        for d in range(NT):
                            nc.tensor.matmul(
                                psa[:],
                                xq[:, d, sp * 128:(sp + 1) * 128],
                                w_sb["v"][:, d, :],
                                start=(d == 0), stop=(d == NT - 1))
                            nc.tensor.matmul(
                                psb[:],
                                xq[:, d, (sp + 1) * 128:(sp + 2) * 128],
                                w_sb["v"][:, d, :],
                                start=(d == 0), stop=(d == NT - 1))
                        nc.vector.tensor_add(v_t[qi * 4 + sp][:], psa[:],
                                             bvb_sb)
                        nc.vector.tensor_add(v_t[qi * 4 + sp + 1][:],
                                             psb[:], bvb_sb)

                    def c_pair(st, dcp):
                        psa = psA.tile([128, SC], f32,
                                       name=f"op{st}{dcp}_{it}", tag="psA")
                        psb = psA.tile([128, SC], f32,
                                       name=f"op{st}{dcp+1}_{it}", tag="psA")
                        for h in range(HPG):
                            lhs = aoT[:, h * S + st * 128:
                                      h * S + (st + 1) * 128]
                            nc.tensor.matmul(
                                psa[:], lhs,
                                wo_sb[:, h, dcp * SC:(dcp + 1) * SC],
                                start=(h == 0), stop=(h == HPG - 1))
                            nc.tensor.matmul(
                                psb[:], lhs,
                                wo_sb[:, h, (dcp + 1) * SC:(dcp + 2) * SC],
                                start=(h == 0), stop=(h == HPG - 1))
                        for dc, op in ((dcp, psa), (dcp + 1, psb)):
                            ot = outpool.tile([128, SC], bf16,
                                              name=f"ot{st}{dc}_{it}",
                                              tag="ot")
                            if dc % 2 == 0:
                                nc.vector.tensor_copy(ot[:], op[:])
                            else:
                                nc.scalar.activation(ot[:], op[:], AF.Copy)
                            nc.sync.dma_start(
                                out[st * 128:(st + 1) * 128,
                                    dc * SC:(dc + 1) * SC], ot[:])

                    def b_block(h, c):
                        """Attention for (head h, query chunk c); yields at
                        filler points (once per score group)."""
                        ntile = 4 * c + 4
                        ngrp = ntile // 2
                        oT = psO.tile([DH, SC], f32, name=f"oT{h}{c}_{it}",
                                      tag="oT")
                        acc = accpool.tile([128, SC], bf16,
                                           name=f"acc{h}{c}_{it}", tag="acc")

                        def emit_pv(at, ts):
                            for j, t_ in enumerate(ts):
                                n0 = max(t_ - 4 * c, 0) * 128
                                nc.tensor.matmul(
                                    oT[:, n0:],
                                    v_t[t_][:, h * DH:(h + 1) * DH],
                                    at[:, j * SC + n0:(j + 1) * SC],
                                    start=(t_ == 0), stop=(t_ == ntile - 1),
                                    skip_group_check=True)

                        pend = []
                        for g in range(ngrp):
                            ss = psS.tile([128, 2 * SC], f32,
                                          name=f"ss{h}{c}{g}_{it}", tag="ss")
                            at = atpool.tile([128, 2 * SC], bf16,
                                             name=f"at{h}{c}{g}_{it}",
                                             tag="at")
                            ts = (2 * g, 2 * g + 1)
                            for j, t_ in enumerate(ts):
                                # full width even for diagonal tiles: keeps
                                # every at element a defined finite value
                                # (masked-out cols are zeroed after exp)
                                nc.tensor.matmul(
                                    ss[:, j * SC:(j + 1) * SC],
                                    kT[h][:, t_ * 128:(t_ + 1) * 128],
                                    qT[h][:, c * SC:(c + 1) * SC],
                                    start=True, stop=True)
                            nc.scalar.activation(at[:], ss[:], AF.Exp,
                                                 bias=0.0, scale=SCALE)
                            for j, t_ in enumerate(ts):
                                rr = t_ - 4 * c
                                if rr >= 0:
                                    nc.vector.tensor_mul(
                                        at[:, j * SC:(j + 1) * SC],
                                        at[:, j * SC:(j + 1) * SC],
                                        msk[rr])
                            if g == 0:
                                nc.vector.tensor_add(acc[:], at[:, 0:SC],
                                                     at[:, SC:2 * SC])
                            else:
                                nc.vector.tensor_add(acc[:], acc[:],
                                                     at[:, 0:SC])
                                nc.vector.tensor_add(acc[:], acc[:],
                                                     at[:, SC:2 * SC])
                            pend.append((at, ts))
                            if len(pend) > 3:
                                yield
                                emit_pv(*pend.pop(0))
                            elif g >= 1:
                                yield
                        while pend:
                            yield
                            emit_pv(*pend.pop(0))
                        dnp = psA.tile([128, SC], f32,
                                       name=f"dn{h}{c}_{it}", tag="psA")
                        nc.tensor.matmul(dnp[:], ones_sb, acc[:],
                                         start=True, stop=True)
                        rec = recpool.tile([128, SC], f32,
                                           name=f"rec{h}{c}_{it}", tag="rec")
                        nc.vector.reciprocal(rec[:], dnp[:])
                        nc.vector.tensor_mul(
                            aoT[:, h * S + c * SC:h * S + (c + 1) * SC],
                            oT[:], rec[:])

                    # ---- prologue: A(0) -------------------------------
                    xq, cos_c, sin_c = load_chunk(0)
                    for cp in (0, 2):
                        a_qk_pair("q", cp, 0, xq, cos_c, sin_c)
                        a_qk_pair("k", cp, 0, xq, cos_c, sin_c)
                    for sp in (0, 2):
                        a_v_pair(sp, 0, xq)

                    # ---- main loop: B(qi) + fillers A(qi+1), C(qi-1) --
                    for qi in range(NQ):
                        a_fill, c_fill = [], []
                        if qi + 1 < NQ:
                            xq2, cos2, sin2 = load_chunk(qi + 1)
                            qi1 = qi + 1
                            for nm in ("q", "k"):
                                for cp in (0, 2):
                                    a_fill.append(
                                        lambda nm=nm, cp=cp, qi1=qi1,
                                        xq2=xq2, cos2=cos2, sin2=sin2:
                                        a_qk_pair(nm, cp, qi1, xq2,
                                                  cos2, sin2))
                            for sp in (0, 2):
                                a_fill.append(
                                    lambda sp=sp, qi1=qi1, xq2=xq2:
                                    a_v_pair(sp, qi1, xq2))
                        # out-projection fillers: B(1)<-C(0), B(3)<-C(1,2)
                        # (B(2) has plenty of A(3) filler; B(3) has no A)
                        C_SRC = {1: (0,), 3: (1, 2)}
                        if phases >= 3:
                            for cc in C_SRC.get(qi, ()):
                                for st in range(4 * cc, 4 * cc + 4):
                                    for dcp in (0, 2):
                                        c_fill.append(
                                            lambda st=st, dcp=dcp: c_pair(
                                                st, dcp))
                        # interleave A and C fillers
                        fillers = []
                        na, ncf = len(a_fill), len(c_fill)
                        ia = ic = 0
                        for k_ in range(na + ncf):
                            if ia * max(ncf, 1) <= ic * max(na, 1) and \
                                    ia < na:
                                fillers.append(a_fill[ia]); ia += 1
                            elif ic < ncf:
                                fillers.append(c_fill[ic]); ic += 1
                            else:
                                fillers.append(a_fill[ia]); ia += 1

                        if phases >= 2:
                            total_pts = HPG * (3, 6, 8, 10)[qi]
                            done = 0
                            pt = 0
                            for h in range(HPG):
                                for _ in b_block(h, qi):
                                    pt += 1
                                    want = min(
                                        len(fillers) * pt // total_pts,
                                        len(fillers))
                                    while done < want:
                                        fillers[done]()
                                        done += 1
                            while done < len(fillers):
                                fillers[done]()
                                done += 1
                        else:
                            for fl in fillers:
                                fl()

                    # ---- tail: out-projection for the last chunk ------
                    if phases >= 3:
                        for st in range(4 * (NQ - 1), 4 * NQ):
                            for dcp in (0, 2):
                                c_pair(st, dcp)
                    else:
                        nc.sync.dma_start(out[0:128, 0:512],
                                          cb16[:, 0:512])
    nc.compile()
    return nc


def host_prep(inputs: dict) -> list:
    """Build per-core input maps (host-side sharding + bf16 relayout)."""
    import ml_dtypes
    bf16 = ml_dtypes.bfloat16

    x = np.asarray(inputs["x"], dtype=np.float32)
    wq = np.asarray(inputs["wq"], dtype=np.float32)
    wk = np.asarray(inputs["wk"], dtype=np.float32)
    wv = np.asarray(inputs["wv"], dtype=np.float32)
    wo = np.asarray(inputs["wo"], dtype=np.float32)
    bq = np.asarray(inputs["bq"], dtype=np.float32)
    bk = np.asarray(inputs["bk"], dtype=np.float32)
    bv = np.asarray(inputs["bv"], dtype=np.float32)

    inv = 1.0 / (10000.0 ** (np.arange(0, DH, 2, dtype=np.float64) / DH))
    ang = np.arange(S, dtype=np.float64)[:, None] * inv[None, :]
    sin = np.repeat(np.sin(ang), 2, axis=1).astype(np.float32)  # [S, DH]
    cos = np.repeat(np.cos(ang), 2, axis=1).astype(np.float32)
    # [NQ, DH, SC]: cosq[qi, p, s] = cos[qi*SC+s, p]
    cosq = np.ascontiguousarray(
        cos.reshape(NQ, SC, DH).transpose(0, 2, 1)).astype(bf16)
    sinq = np.ascontiguousarray(
        sin.reshape(NQ, SC, DH).transpose(0, 2, 1)).astype(bf16)

    P = np.zeros((DH, DH), np.float32)
    idx = np.arange(0, DH, 2)
    P[idx, idx + 1] = -1.0    # out[2i]   = -x[2i+1]
    P[idx + 1, idx] = 1.0     # out[2i+1] =  x[2i]
    PT = np.ascontiguousarray(P.T)

    # packed bf16 consts: PT | ones | mask0..3
    # mask_rr (for diagonal tile with row offset rr): cols < rr*128 -> 0,
    # cols in the rr block -> lower triangle (keep col >= row), rest -> 1.
    tri = (np.arange(128)[None, :] >= np.arange(128)[:, None])
    masks = []
    for rr in range(4):
        m = np.ones((128, SC), np.float32)
        m[:, :rr * 128] = 0.0
        m[:, rr * 128:(rr + 1) * 128] = tri.astype(np.float32)
        masks.append(m)
    cb16 = np.concatenate(
        [PT, np.ones((128, 128), np.float32)] + masks,
        axis=1).astype(bf16)

    # [NQ, 128, NT, SC]: xTq[qi, p, d, s] = x[b][qi*SC+s, d*128+p]
    xTqb = [np.ascontiguousarray(
        x[b].reshape(NQ, SC, NT, 128).transpose(0, 3, 2, 1)).astype(bf16)
        for b in range(B)]

    in_maps = []
    for core in range(N_CORES):
        b, g = divmod(core, G)
        c0 = g * CW
        # [128, NT, CW]: wqb[p, d, c] = wq[c0+c, d*128+p]
        wqb = np.ascontiguousarray(
            wq[c0:c0 + CW, :].reshape(CW, NT, 128).transpose(2, 1, 0)
        ).astype(bf16)
        wkb = np.ascontiguousarray(
            wk[c0:c0 + CW, :].reshape(CW, NT, 128).transpose(2, 1, 0)
        ).astype(bf16)
        wvb = np.ascontiguousarray(
            wv[c0:c0 + CW, :].reshape(CW, NT, 128).transpose(2, 1, 0)
        ).astype(bf16)
        # [128, HPG, D]: wob[p, h, j] = wo[j, c0+h*128+p]
        wob = np.ascontiguousarray(
            wo[:, c0:c0 + CW].reshape(D, HPG, 128).transpose(2, 1, 0)
        ).astype(bf16)
        # packed f32 consts: bvb (broadcast) | bq columns | bk columns
        cf32 = np.zeros((128, CW + 2 * HPG), np.float32)
        cf32[:, 0:CW] = bv[c0:c0 + CW][None, :]
        cf32[:, CW:CW + HPG] = bq[c0:c0 + CW].reshape(HPG, DH).T
        cf32[:, CW + HPG:] = bk[c0:c0 + CW].reshape(HPG, DH).T
        in_maps.append({
            "xTq": xTqb[b],
            "wqb": wqb,
            "wkb": wkb,
            "wvb": wvb,
            "wob": wob,
            "cf32d": cf32,
            "cb16d": cb16,
            "cosq": cosq,
            "sinq": sinq,
        })
    return in_maps


def _get_nc():
    if "nc" not in _NC_CACHE:
        _NC_CACHE["nc"] = build_attn_nc(iters=1)
    return _NC_CACHE["nc"]


def kernel(**inputs) -> np.ndarray:
    from concourse.bass_utils import run_bass_kernel_spmd

    nc = _get_nc()
    in_maps = host_prep(inputs)
    res = run_bass_kernel_spmd(nc, in_maps, core_ids=list(range(N_CORES)))
    bo = np.asarray(inputs["bo"], dtype=np.float32)
    outp = np.zeros((B, S, D), np.float32)
    for core in range(N_CORES):
        outp[core // G] += np.asarray(res.results[core]["out"],
                                      dtype=np.float32)
    outp += bo[None, None, :]
    return outp
